# revision 17
# baseline (speedup 1.0000x reference)
"""Gemma2 fused attention (B=1, S=4096, HID=2304, NH=8, NKV=4, HD=256,
sliding window 2048, softcap 50) on 8 Trainium2 NeuronCores.

Sharding: one query head per core (its GQA kv head recomputed per core);
o_proj is sharded over the contraction dim, per-core partials are summed
on the host.

Per-core math (core c, head h=c, kv group g=c//2):
  qT,kT = (W @ X.T) in [head_dim, tok] layout, RoPE'd on device (cos/sin
  tables precomputed on host; attention scale folded into Wq exactly).
  v in [tok, head_dim] layout.
  S.T[k,q] = kT.T @ qT; u = tanh(S.T/50); E = exp(50*u + mask) in bf16
  (softcap bounds logits to +-50 so no max-subtraction is needed).
  Mask handled per 128(k) x 512(q) block: all-zero blocks skip the add,
  fully-masked blocks are skipped entirely, mixed blocks add mask*0.02
  from a host-packed block stack (data-driven, no pattern assumption).
  Z = ones.T @ E (PSUM row), attnT = (E @ v).T via lhsT=v chunks.
  out_partial[tok, 2304] = attnT.T @ WoT with 1/Z fused into the
  PSUM->SBUF copy. Host sums the 8 partials.
"""

import numpy as np
import ml_dtypes
from contextlib import ExitStack

import concourse.bass as bass
import concourse.tile as tile
import concourse.mybir as mybir
from concourse.bass_utils import run_bass_kernel_spmd
from concourse.masks import make_identity
from concourse.vector_clock import ScopedClock

N_CORES = 8
HID = 2304
NH, NKV, HD = 8, 4, 256
SCALE = 256.0 ** -0.5
SOFTCAP = 50.0
ROPE_THETA = 10000.0
KC = HID // 128  # 18 contraction chunks for the projections
CC_GROUPS = [[0, 1], [2, 3], [4, 5], [6, 7]]  # GQA pair exchange

BF16 = mybir.dt.bfloat16
F32 = mybir.dt.float32
AF = mybir.ActivationFunctionType

TRACE = False  # test harness flips this to get NTFF exec time


class TC(tile.TileContext):
    """TileContext whose final drain splits sem waits one-per-instruction
    (this walrus rejects instructions carrying more than one wait)."""

    def _drain_and_barrier(self, tick_clock, wait_clock):
        probe = self.nc.sync.nop(nofuse=True, hint="drain_waits")
        wait_clock.add_sem_waits(
            probe.ins, ScopedClock({None: tick_clock.global_clock})
        )
        waits = list(probe.ins.sync_info.on_wait)
        probe.ins.sync_info.on_wait = waits[:1]
        rest = waits[1:]
        while rest:
            extra = self.nc.sync.nop(nofuse=True, hint="drain_waits")
            extra.ins.sync_info = mybir.SyncInfo(on_wait=rest[:1], on_update=[])
            rest = rest[1:]
        self.nc.sync.drain()
        self.nc.all_engine_barrier()
        popped = self.nc._tile_sem_poison_stack.pop()
        assert popped is self._sem_poison
        self.nc.clear_and_free_semaphores(list(self.sems.allocated().values()))
        self.nc.all_engine_barrier()


def split_multi_waits(nc):
    """Split multi-wait instructions: extras move onto same-engine NoOps
    inserted immediately before (engines execute in program order)."""
    ctr = 0
    for f in nc.m.functions:
        for b in f.blocks:
            insts = list(b.instructions)
            new = []
            changed = False
            for inst in insts:
                si = inst.sync_info
                if si is not None and len(si.on_wait) > 1:
                    waits = list(si.on_wait)
                    for w in waits[:-1]:
                        ctr += 1
                        nop = mybir.InstNoOp(
                            name=f"I-waitsplit-{ctr}",
                            engine=inst.engine,
                            debug=inst.debug,
                            sync_info=mybir.SyncInfo(on_wait=[w], on_update=[]),
                        )
                        new.append(nop)
                    inst.sync_info = mybir.SyncInfo(
                        on_wait=[waits[-1]], on_update=list(si.on_update)
                    )
                    changed = True
                new.append(inst)
            if changed:
                b.instructions = new
    return ctr


def _classify_mask(mask, S):
    """Per (k-chunk 128, q-block 512) block: 'skip' (fully masked),
    'clean' (all zero) or mixed (apply additively). Each plan entry is
    (j, mix, q0, q1): only q-columns [q0, q1) have any unmasked k in the
    chunk, so S/E/attnT work is restricted to that slice. The first entry
    of every row is full-width so it can init the PSUM accumulation and
    zacc. Returns plan and the packed mixed-block stack (already scaled
    by 1/SOFTCAP)."""
    maskT = np.ascontiguousarray(np.asarray(mask, np.float32)[0, 0].T)  # [k, q]
    nj, nq = S // 128, S // 512
    blocks = maskT.reshape(nj, 128, nq, 512)
    mx = blocks.max(axis=(1, 3))
    mn = blocks.min(axis=(1, 3))
    skip = mx < -1e8
    clean = (mx == 0.0) & (mn == 0.0)
    plan = []
    mix_blocks = []
    for qb in range(nq):
        row = []
        for j in range(nj):
            if skip[j, qb]:
                continue
            if clean[j, qb]:
                row.append((j, -1, 0, 512))
            else:
                blk = maskT[j * 128:(j + 1) * 128, qb * 512:(qb + 1) * 512]
                col_ok = blk.max(axis=0) > -1e8
                q0 = int(col_ok.argmax())
                q1 = 512 - int(col_ok[::-1].argmax())
                if not col_ok[q0:q1].all():
                    q0, q1 = 0, 512  # non-contiguous valid span: no trim
                mix_blocks.append((blk * (1.0 / SOFTCAP)).astype(np.float32))
                row.append((j, len(mix_blocks) - 1, q0, q1))
        if not row:
            # fully-masked q-block (unreachable for causal masks): keep the
            # diagonal chunks so the PSUM accumulations are still defined
            for j in range(4 * qb, 4 * qb + 4):
                mix_blocks.append(
                    (maskT[j * 128:(j + 1) * 128, qb * 512:(qb + 1) * 512]
                     * (1.0 / SOFTCAP)).astype(np.float32))
                row.append((j, len(mix_blocks) - 1, 0, 512))
        # first entry must be full-width (inits PSUM + zacc). Prefer the
        # lowest-j full-width entry (oldest k/v data, longest ready) so the
        # high-j chunks — which depend on the freshest k/v exchange — run
        # last; else widen the first (safe for mixed entries: the additive
        # mask zeroes E outside the valid span).
        full = next((i for i, e in enumerate(row)
                     if e[2] == 0 and e[3] == 512), None)
        if full is None:
            j, mix, _, _ = row[0]
            assert mix >= 0
            row[0] = (j, mix, 0, 512)
        else:
            row = [row[full]] + row[:full] + row[full + 1:]
        plan.append(row)
    if mix_blocks:
        maskb = np.stack(mix_blocks)
    else:
        maskb = np.zeros((1, 128, 512), np.float32)
    return plan, maskb


def _build(S, plan, nmix):
    """Emit the SPMD program (identical for all cores; only data differs)."""
    NT = S // 512  # token/query 512-blocks
    nc = bass.Bass("TRN2", target_bir_lowering=False, debug=False,
                   num_devices=N_CORES)

    xt_d = nc.dram_tensor("xt", [HID, S], BF16, kind="ExternalInput")
    wqk_d = nc.dram_tensor("wqk", [HID, 512], BF16, kind="ExternalInput")
    wo_d = nc.dram_tensor("wo", [256, HID], BF16, kind="ExternalInput")
    cosq_d = nc.dram_tensor("cosq", [128, S], F32, kind="ExternalInput")
    sinq_d = nc.dram_tensor("sinq", [128, S], F32, kind="ExternalInput")
    coss_d = nc.dram_tensor("coss", [128, S], F32, kind="ExternalInput")
    sins_d = nc.dram_tensor("sins", [128, S], F32, kind="ExternalInput")
    maskb_d = nc.dram_tensor("maskb", [nmix, 128, 512], F32,
                             kind="ExternalInput")
    out_d = nc.dram_tensor("out", [S, HID], F32, kind="ExternalOutput")
    # pairwise k/v exchange: each core projects q plus ONE of (k, v) for
    # its GQA group ("s", in [hd, tok] layout); the pair AllGather makes
    # both halves visible as cc_out[T] = [k(2x128x512) | v(2x128x512)]
    # identically on both cores (rank order), so the consuming program is
    # parity-independent.
    cc_in = nc.dram_tensor("cc_in", [NT, 2, 128, 512], BF16, kind="Internal")
    cc_out = nc.dram_tensor("cc_out", [NT, 2, 2, 128, 512], BF16,
                            kind="Internal")

    with ExitStack() as ctx:
        tc = ctx.enter_context(TC(nc))
        P = lambda name, bufs, space="SBUF": ctx.enter_context(
            tc.tile_pool(name=name, bufs=bufs, space=space))

        wpool = P("w", 1)
        xpool = P("x", 2)
        cspool = P("cs", 2)
        qkpool = P("qk", 1)
        vpool = P("v", 1)
        spool = P("s", 2)
        vspool = P("vs", 2)
        tmppool = P("tmp", 4)
        upool = P("u", 4)
        epool = P("e", 10)
        mpool = P("m", 3)
        apool = P("a", 1)
        zpool = P("z", 2)
        zapool = P("za", 2)
        opool = P("o", 4)
        rpool = P("r", 1)
        dpool = P("d", 1, "DRAM")

        ps_qk = P("ps_qk", 2, "PSUM")
        ps_v = P("ps_v", 1, "PSUM")  # shared: proj tiles + transpose pads
        ps_s = P("ps_s", 3, "PSUM")
        ps_o = P("ps_o", 2, "PSUM")

        # --- resident weights / constants (wqk streams in chunk-
        # interleaved inside phase A block 0) ---
        wqk = wpool.tile([128, KC * 512], BF16, tag="wqk")
        wo = wpool.tile([128, 2 * HID], BF16, tag="wo")
        nc.sync.dma_start(
            wo[:].rearrange("p (c f) -> p c f", f=HID),
            wo_d[:, :].rearrange("(c p) f -> p c f", p=128))
        ones = wpool.tile([128, 1], BF16, tag="ones")
        nc.gpsimd.memset(ones[:], 1.0)
        ident = wpool.tile([128, 128], BF16, tag="ident")
        make_identity(nc, ident[:])

        # persistent activations (bf16, [128, S] each)
        qlo = qkpool.tile([128, S], BF16, tag="qlo")
        qhi = qkpool.tile([128, S], BF16, tag="qhi")
        klo = qkpool.tile([128, S], BF16, tag="klo")
        khi = qkpool.tile([128, S], BF16, tag="khi")
        vt = vpool.tile([128, (S // 128) * 256], BF16, tag="vt")
        alo = apool.tile([128, S], BF16, tag="alo")
        ahi = apool.tile([128, S], BF16, tag="ahi")
        rc = rpool.tile([128, S // 128], F32, tag="rc")
        zc = rpool.tile([128, S // 128], F32, tag="zc")
        zdram = dpool.tile([NT, 512], F32, tag="zdram")

        def phase_a(T):
            """Return emission units (closures) for QKV block T: project
            q + s (s = k on even cores, v on odd — distinguished purely by
            input data: the wqk second half and the coss/sins tables, which
            are identity for v), then exchange s within the core pair and
            transpose the received v into [tok, hd] layout."""
            c0 = T * 512
            xt = xpool.tile([128, KC * 512], BF16, tag="xt")
            slo = spool.tile([128, 512], BF16, tag="slo")
            shi = spool.tile([128, 512], BF16, tag="shi")
            qk_dst = [(qlo, c0), (qhi, c0), (slo, 0), (shi, 0)]
            units = []

            def dma_unit():
                if T == 0:
                    # chunk-interleaved so the first matmul only waits for
                    # chunk 0, not the whole 4.5 MB of weights+activations
                    for kc in range(KC):
                        nc.sync.dma_start(
                            wqk[:, kc * 512:(kc + 1) * 512],
                            wqk_d[kc * 128:(kc + 1) * 128, :])
                        nc.sync.dma_start(
                            xt[:, kc * 512:(kc + 1) * 512],
                            xt_d[kc * 128:(kc + 1) * 128, c0:c0 + 512])
                else:
                    nc.sync.dma_start(
                        xt[:].rearrange("p (c s) -> p c s", s=512),
                        xt_d[:, c0:c0 + 512].rearrange("(c p) s -> p c s",
                                                       p=128))
            units.append(dma_unit)

            cosq = cspool.tile([128, 512], F32, tag="cosq")
            sinq = cspool.tile([128, 512], F32, tag="sinq")
            coss = cspool.tile([128, 512], F32, tag="coss")
            sins = cspool.tile([128, 512], F32, tag="sins")
            cs_pair = [(cosq, sinq), (coss, sins)]

            def cs_unit():
                nc.sync.dma_start(cosq[:], cosq_d[:, c0:c0 + 512])
                nc.sync.dma_start(sinq[:], sinq_d[:, c0:c0 + 512])
                nc.sync.dma_start(coss[:], coss_d[:, c0:c0 + 512])
                nc.sync.dma_start(sins[:], sins_d[:, c0:c0 + 512])
            units.append(cs_unit)

            def rope_pair(plo, phi, pair):
                cos, sin = cs_pair[pair]
                (dlo, o), (dhi, _) = qk_dst[2 * pair], qk_dst[2 * pair + 1]
                t1 = tmppool.tile([128, 512], F32, tag="tmp")
                nc.vector.tensor_mul(t1[:], phi[:], sin[:])
                t2 = tmppool.tile([128, 512], F32, tag="tmp")
                nc.vector.tensor_mul(t2[:], plo[:], cos[:])
                nc.vector.tensor_sub(dlo[:, o:o + 512], t2[:], t1[:])
                t3 = tmppool.tile([128, 512], F32, tag="tmp")
                nc.vector.tensor_mul(t3[:], plo[:], sin[:])
                t4 = tmppool.tile([128, 512], F32, tag="tmp")
                nc.vector.tensor_mul(t4[:], phi[:], cos[:])
                nc.vector.tensor_add(dhi[:, o:o + 512], t4[:], t3[:])

            pp = {}

            def qk_unit(ft):
                ps = ps_qk.tile([128, 512], F32, tag="ps_qk")
                for kc in range(KC):
                    nc.tensor.matmul(
                        ps[:],
                        wqk[:, kc * 512 + ft * 128: kc * 512 + ft * 128 + 128],
                        xt[:, kc * 512:(kc + 1) * 512],
                        start=(kc == 0), stop=(kc == KC - 1))
                pp[ft] = ps
                if ft % 2 == 1:  # rotate the (lo, hi) pair
                    rope_pair(pp[ft - 1], pp[ft], ft // 2)

            def qk_chunk_major():
                # block 0 is paced by the weight/activation DMAs: keep 4
                # accumulations in flight (borrowing idle B-phase banks) so
                # each arriving chunk feeds 4 matmuls
                psA0 = ps_qk.tile([128, 512], F32, tag="ps_qk")
                psA1 = ps_qk.tile([128, 512], F32, tag="ps_qk")
                psA2 = ps_s.tile([128, 512], F32, tag="ps_s")
                psA3 = ps_o.tile([128, 512], F32, tag="ps_o")
                psA = [psA0, psA1, psA2, psA3]
                for kc in range(KC):
                    for ft in range(4):
                        nc.tensor.matmul(
                            psA[ft][:],
                            wqk[:, kc * 512 + ft * 128: kc * 512 + ft * 128 + 128],
                            xt[:, kc * 512:(kc + 1) * 512],
                            start=(kc == 0), stop=(kc == KC - 1))
                for pair in range(2):
                    rope_pair(psA[2 * pair], psA[2 * pair + 1], pair)

            if T == 0:
                units.append(qk_chunk_major)
            else:
                for ft in range(4):
                    units.append(lambda ft=ft: qk_unit(ft))

            def send_unit():
                nc.sync.dma_start(cc_in[T, 0, :, :], slo[:])
                nc.sync.dma_start(cc_in[T, 1, :, :], shi[:])

            def cc_unit():
                nc.gpsimd.collective_compute(
                    "AllGather", mybir.AluOpType.bypass, CC_GROUPS,
                    ins=[cc_in[T, :, :, :]], outs=[cc_out[T, :, :, :, :]])

            vsl = vspool.tile([128, 512], BF16, tag="vsl")
            vsh = vspool.tile([128, 512], BF16, tag="vsh")

            def recv_unit():
                nc.sync.dma_start(klo[:, c0:c0 + 512], cc_out[T, 0, 0, :, :])
                nc.sync.dma_start(khi[:, c0:c0 + 512], cc_out[T, 0, 1, :, :])
                nc.sync.dma_start(vsl[:], cc_out[T, 1, 0, :, :])
                nc.sync.dma_start(vsh[:], cc_out[T, 1, 1, :, :])

            def tr_unit(sub):
                tok = T * 4 + sub
                tp = ps_v.tile([128, 256], BF16, tag="ps_v")
                nc.tensor.transpose(tp[:, 0:128],
                                    vsl[:, sub * 128:(sub + 1) * 128],
                                    ident[:])
                nc.tensor.transpose(tp[:, 128:256],
                                    vsh[:, sub * 128:(sub + 1) * 128],
                                    ident[:])
                nc.vector.tensor_copy(vt[:, tok * 256:(tok + 1) * 256], tp[:])

            units.append(send_unit)
            units.append(cc_unit)
            units.append(recv_unit)
            for sub in range(4):
                units.append(lambda sub=sub: tr_unit(sub))
            return units

        def phase_b(qb):
            """Return emission units for attention q-block qb, one per
            k-chunk. The S matmuls of j lead the E-consumers of j-1 so the
            ACT chain has a full PE iteration of slack."""
            c0 = qb * 512
            zacc = zapool.tile([128, 512], F32, tag="za")
            olo = ps_o.tile([128, 512], F32, tag="ps_o")
            ohi = ps_o.tile([128, 512], F32, tag="ps_o")
            row = plan[qb]
            state = {}

            def s_unit(idx):
                j, mix, q0, q1 = row[idx]
                w = q1 - q0
                if mix >= 0:
                    mk = mpool.tile([128, 512], F32, tag="m")
                    nc.sync.dma_start(mk[:, :w], maskb_d[mix, :, q0:q1])
                else:
                    mk = None
                sps = ps_s.tile([128, 512], F32, tag="ps_s")
                nc.tensor.matmul(sps[:, q0:q1], klo[:, j * 128:(j + 1) * 128],
                                 qlo[:, c0 + q0:c0 + q1],
                                 start=True, stop=False)
                nc.tensor.matmul(sps[:, q0:q1], khi[:, j * 128:(j + 1) * 128],
                                 qhi[:, c0 + q0:c0 + q1],
                                 start=False, stop=True)
                e = epool.tile([128, 512], BF16, tag="e")
                u = upool.tile([128, 512], F32, tag="u")
                nc.scalar.activation(u[:, :w], sps[:, q0:q1], AF.Tanh,
                                     scale=1.0 / SOFTCAP)
                if mk is not None:
                    u2 = upool.tile([128, 512], F32, tag="u")
                    nc.vector.tensor_add(u2[:, :w], u[:, :w], mk[:, :w])
                    u = u2
                nc.scalar.activation(e[:, q0:q1], u[:, :w], AF.Exp,
                                     scale=SOFTCAP)
                if idx == 0:
                    nc.vector.tensor_copy(zacc[:], e[:])
                else:
                    nc.vector.tensor_add(zacc[:, q0:q1], zacc[:, q0:q1],
                                         e[:, q0:q1])
                state[idx] = e

            def mm_unit(idx):
                j, _, q0, q1 = row[idx]
                e = state.pop(idx)
                first, last = idx == 0, idx == len(row) - 1
                nc.tensor.matmul(olo[:, q0:q1], vt[:, j * 256:j * 256 + 128],
                                 e[:, q0:q1], start=first, stop=last)
                nc.tensor.matmul(ohi[:, q0:q1],
                                 vt[:, j * 256 + 128:(j + 1) * 256],
                                 e[:, q0:q1], start=first, stop=last)

            def tail_unit():
                nc.vector.tensor_copy(alo[:, c0:c0 + 512], olo[:])
                nc.vector.tensor_copy(ahi[:, c0:c0 + 512], ohi[:])
                # single f32->bf16 rounding of zacc so the 128-partition
                # reduction runs as a 1-cycle/row bf16 matmul (fp32 is 4x)
                zaccb = zpool.tile([128, 512], BF16, tag="zb")
                nc.scalar.copy(zaccb[:], zacc[:])
                zps = ps_s.tile([1, 512], F32, tag="ps_s")
                nc.tensor.matmul(zps[:], ones[:], zaccb[:],
                                 start=True, stop=True)
                zrow = zpool.tile([1, 512], F32, tag="z")
                nc.vector.tensor_copy(zrow[:], zps[:])
                nc.sync.dma_start(zdram[qb, :], zrow[:])
                # incremental 1/Z so proj for this q-block can start now
                nc.sync.dma_start(
                    zc[:, 4 * qb:4 * qb + 4],
                    zdram[qb, :].rearrange("(b p) -> p b", p=128))
                nc.vector.reciprocal(rc[:, 4 * qb:4 * qb + 4],
                                     zc[:, 4 * qb:4 * qb + 4])

            units = [lambda: s_unit(0)]
            for idx in range(1, len(row)):
                units.append(lambda idx=idx: (s_unit(idx), mm_unit(idx - 1)))
            units.append(lambda: (mm_unit(len(row) - 1), tail_unit()))
            return units

        # PE warmup: a few throwaway matmuls so HAM reaches 8/8 before
        # the first real accumulation
        scratch = wpool.tile([128, 512], BF16, tag="scratch")
        nc.gpsimd.memset(scratch[:], 0.0)
        wps = ps_s.tile([128, 512], F32, tag="ps_s")
        for _ in range(12):
            nc.tensor.matmul(wps[:], scratch[:, :128], scratch[:],
                             start=True, stop=True)

        # output projection units (one per (tok-tile, feat-block)); the
        # 1/Z normalization is fused into the PSUM->SBUF copy. These are
        # woven into later B phases so the 37 MB output DMA spreads over
        # the whole kernel instead of saturating the tail.
        fbs = [(0, 512), (512, 512), (1024, 512), (1536, 512), (2048, 256)]

        def proj_unit(t, fi):
            f0, fw = fbs[fi]
            pool = ps_qk if fi % 3 < 2 else ps_v
            ps = pool.tile([128, 512], F32, tag=pool.name)
            nc.tensor.matmul(ps[:, :fw], alo[:, t * 128:(t + 1) * 128],
                             wo[:, f0:f0 + fw], start=True, stop=False)
            nc.tensor.matmul(ps[:, :fw], ahi[:, t * 128:(t + 1) * 128],
                             wo[:, HID + f0:HID + f0 + fw],
                             start=False, stop=True)
            osb = opool.tile([128, 512], F32, tag="o")
            if fi % 2 == 0:
                nc.scalar.activation(osb[:, :fw], ps[:, :fw], AF.Copy,
                                     scale=rc[:, t:t + 1])
            else:
                nc.vector.tensor_scalar_mul(osb[:, :fw], ps[:, :fw],
                                            rc[:, t:t + 1])
            nc.sync.dma_start(out_d[t * 128:(t + 1) * 128, f0:f0 + fw],
                              osb[:, :fw])

        def phase_c(qb):
            return [lambda t=t, fi=fi: proj_unit(t, fi)
                    for t in range(4 * qb, 4 * qb + 4)
                    for fi in range(len(fbs))]

        def weave(bunits, aunits):
            """Alternate B and A units so stalled B consumers never block
            independent A matmuls in the in-order PE queue."""
            out = []
            na, nb = len(aunits), len(bunits)
            ai = 0
            for bi, bu in enumerate(bunits):
                out.append(bu)
                want = (bi + 1) * na // nb
                while ai < want:
                    out.append(aunits[ai])
                    ai += 1
            out.extend(aunits[ai:])
            return out

        for u in phase_a(0):
            u()
        for T in range(NT):
            bunits = phase_b(T)
            aunits = phase_a(T + 1) if T + 1 < NT else []
            if T >= 1:
                aunits = aunits + phase_c(T - 1)
            with nc.named_scope(f"B{T}"):
                for u in weave(bunits, aunits):
                    u()
        with nc.named_scope("Ctail"):
            for qb in (NT - 1,):
                for u in phase_c(qb):
                    u()


    split_multi_waits(nc)
    return nc


def kernel(hidden_states, attention_mask, position_ids, Wqkv, Wo):
    bf16 = ml_dtypes.bfloat16
    hidden = np.asarray(hidden_states, np.float32)
    S = hidden.shape[1]
    X = hidden[0]  # [S, HID]
    XT = np.ascontiguousarray(X.T).astype(bf16)  # [HID, S]

    pos = np.asarray(position_ids)[0].astype(np.float64)
    inv = 1.0 / (ROPE_THETA ** (np.arange(0, HD, 2, dtype=np.float64) / HD))
    freqs = inv[:, None] * pos[None, :]  # [128, S]
    cosT = np.cos(freqs).astype(np.float32)
    sinT = np.sin(freqs).astype(np.float32)

    plan, maskb = _classify_mask(attention_mask, S)

    Wqkv = np.asarray(Wqkv, np.float32)
    Wo = np.asarray(Wo, np.float32)

    one_cs = np.ones_like(cosT)
    zero_cs = np.zeros_like(sinT)

    in_maps = []
    for c in range(N_CORES):
        g = c // (NH // NKV)
        wq = Wqkv[c * HD:(c + 1) * HD] * SCALE  # exact: SCALE = 2**-4
        wk = Wqkv[NH * HD + g * HD: NH * HD + (g + 1) * HD]
        wv = Wqkv[(NH + NKV) * HD + g * HD: (NH + NKV) * HD + (g + 1) * HD]
        # even core of a pair projects+sends k (RoPE'd), odd projects+sends
        # v (coss=1/sins=0 makes the s-RoPE the identity)
        ws = wk if c % 2 == 0 else wv
        wqk = np.ascontiguousarray(
            np.concatenate([wq.T, ws.T], axis=1)).astype(bf16)
        wot = np.ascontiguousarray(Wo[:, c * HD:(c + 1) * HD].T).astype(bf16)
        in_maps.append({
            "xt": XT, "wqk": wqk, "wo": wot,
            "cosq": cosT, "sinq": sinT,
            "coss": cosT if c % 2 == 0 else one_cs,
            "sins": sinT if c % 2 == 0 else zero_cs,
            "maskb": maskb,
        })

    nc = _build(S, plan, maskb.shape[0])
    res = run_bass_kernel_spmd(nc, in_maps, list(range(N_CORES)),
                               trace=TRACE)
    out = res.results[0]["out"].astype(np.float64)
    for c in range(1, N_CORES):
        out += res.results[c]["out"]
    kernel.last_exec_time_ns = res.exec_time_ns
    kernel.last_results = res
    return out[None].astype(np.float32)


kernel.last_exec_time_ns = None
kernel.last_results = None



# revision 19
# speedup vs baseline: 1.2229x; 1.2229x over previous
"""Gemma2 fused attention (B=1, S=4096, HID=2304, NH=8, NKV=4, HD=256,
sliding window 2048, softcap 50) on 8 Trainium2 NeuronCores.

Sharding: one query head per core (its GQA kv head recomputed per core);
o_proj is sharded over the contraction dim, per-core partials are summed
on the host.

Per-core math (core c, head h=c, kv group g=c//2):
  qT,kT = (W @ X.T) in [head_dim, tok] layout, RoPE'd on device (cos/sin
  tables precomputed on host; attention scale folded into Wq exactly).
  v in [tok, head_dim] layout.
  S.T[k,q] = kT.T @ qT; u = tanh(S.T/50); E = exp(50*u + mask) in bf16
  (softcap bounds logits to +-50 so no max-subtraction is needed).
  Mask handled per 128(k) x 512(q) block: all-zero blocks skip the add,
  fully-masked blocks are skipped entirely, mixed blocks add mask*0.02
  from a host-packed block stack (data-driven, no pattern assumption).
  Z = ones.T @ E (PSUM row), attnT = (E @ v).T via lhsT=v chunks.
  out_partial[tok, 2304] = attnT.T @ WoT with 1/Z fused into the
  PSUM->SBUF copy. Host sums the 8 partials.
"""

import numpy as np
import ml_dtypes
from contextlib import ExitStack

import concourse.bass as bass
import concourse.tile as tile
import concourse.mybir as mybir
from concourse.bass_utils import run_bass_kernel_spmd
from concourse.masks import make_identity
from concourse.vector_clock import ScopedClock

N_CORES = 8
HID = 2304
NH, NKV, HD = 8, 4, 256
SCALE = 256.0 ** -0.5
SOFTCAP = 50.0
ROPE_THETA = 10000.0
KC = HID // 128  # 18 contraction chunks for the projections
CC_GROUPS = [[0, 1], [2, 3], [4, 5], [6, 7]]  # GQA pair exchange

BF16 = mybir.dt.bfloat16
F32 = mybir.dt.float32
AF = mybir.ActivationFunctionType

TRACE = False  # test harness flips this to get NTFF exec time


class TC(tile.TileContext):
    """TileContext whose final drain splits sem waits one-per-instruction
    (this walrus rejects instructions carrying more than one wait)."""

    def _drain_and_barrier(self, tick_clock, wait_clock):
        probe = self.nc.sync.nop(nofuse=True, hint="drain_waits")
        wait_clock.add_sem_waits(
            probe.ins, ScopedClock({None: tick_clock.global_clock})
        )
        waits = list(probe.ins.sync_info.on_wait)
        probe.ins.sync_info.on_wait = waits[:1]
        rest = waits[1:]
        while rest:
            extra = self.nc.sync.nop(nofuse=True, hint="drain_waits")
            extra.ins.sync_info = mybir.SyncInfo(on_wait=rest[:1], on_update=[])
            rest = rest[1:]
        self.nc.sync.drain()
        self.nc.all_engine_barrier()
        popped = self.nc._tile_sem_poison_stack.pop()
        assert popped is self._sem_poison
        self.nc.clear_and_free_semaphores(list(self.sems.allocated().values()))
        self.nc.all_engine_barrier()


def split_multi_waits(nc):
    """Split multi-wait instructions: extras move onto same-engine NoOps
    inserted immediately before (engines execute in program order)."""
    ctr = 0
    for f in nc.m.functions:
        for b in f.blocks:
            insts = list(b.instructions)
            new = []
            changed = False
            for inst in insts:
                si = inst.sync_info
                if si is not None and len(si.on_wait) > 1:
                    waits = list(si.on_wait)
                    for w in waits[:-1]:
                        ctr += 1
                        nop = mybir.InstNoOp(
                            name=f"I-waitsplit-{ctr}",
                            engine=inst.engine,
                            debug=inst.debug,
                            sync_info=mybir.SyncInfo(on_wait=[w], on_update=[]),
                        )
                        new.append(nop)
                    inst.sync_info = mybir.SyncInfo(
                        on_wait=[waits[-1]], on_update=list(si.on_update)
                    )
                    changed = True
                new.append(inst)
            if changed:
                b.instructions = new
    return ctr


def _classify_mask(mask, S):
    """Per (k-chunk 128, q-block 512) block: 'skip' (fully masked),
    'clean' (all zero) or mixed (apply additively). Each plan entry is
    (j, mix, q0, q1): only q-columns [q0, q1) have any unmasked k in the
    chunk, so S/E/attnT work is restricted to that slice. The first entry
    of every row is full-width so it can init the PSUM accumulation and
    zacc. Returns plan and the packed mixed-block stack (already scaled
    by 1/SOFTCAP)."""
    maskT = np.ascontiguousarray(np.asarray(mask, np.float32)[0, 0].T)  # [k, q]
    nj, nq = S // 128, S // 512
    blocks = maskT.reshape(nj, 128, nq, 512)
    mx = blocks.max(axis=(1, 3))
    mn = blocks.min(axis=(1, 3))
    skip = mx < -1e8
    clean = (mx == 0.0) & (mn == 0.0)
    plan = []
    mix_blocks = []
    for qb in range(nq):
        row = []
        for j in range(nj):
            if skip[j, qb]:
                continue
            if clean[j, qb]:
                row.append((j, -1, 0, 512))
            else:
                blk = maskT[j * 128:(j + 1) * 128, qb * 512:(qb + 1) * 512]
                col_ok = blk.max(axis=0) > -1e8
                q0 = int(col_ok.argmax())
                q1 = 512 - int(col_ok[::-1].argmax())
                if not col_ok[q0:q1].all():
                    q0, q1 = 0, 512  # non-contiguous valid span: no trim
                mix_blocks.append((blk * (1.0 / SOFTCAP)).astype(np.float32))
                row.append((j, len(mix_blocks) - 1, q0, q1))
        if not row:
            # fully-masked q-block (unreachable for causal masks): keep the
            # diagonal chunks so the PSUM accumulations are still defined
            for j in range(4 * qb, 4 * qb + 4):
                mix_blocks.append(
                    (maskT[j * 128:(j + 1) * 128, qb * 512:(qb + 1) * 512]
                     * (1.0 / SOFTCAP)).astype(np.float32))
                row.append((j, len(mix_blocks) - 1, 0, 512))
        # first entry must be full-width (inits PSUM + zacc). Prefer the
        # lowest-j full-width entry (oldest k/v data, longest ready) so the
        # high-j chunks — which depend on the freshest k/v exchange — run
        # last; else widen the first (safe for mixed entries: the additive
        # mask zeroes E outside the valid span).
        full = next((i for i, e in enumerate(row)
                     if e[2] == 0 and e[3] == 512), None)
        if full is None:
            j, mix, _, _ = row[0]
            assert mix >= 0
            row[0] = (j, mix, 0, 512)
        else:
            row = [row[full]] + row[:full] + row[full + 1:]
        plan.append(row)
    if mix_blocks:
        maskb = np.stack(mix_blocks)
    else:
        maskb = np.zeros((1, 128, 512), np.float32)
    return plan, maskb


def _build(S, plan, nmix):
    """Emit the SPMD program (identical for all cores; only data differs)."""
    NT = S // 512  # token/query 512-blocks
    nc = bass.Bass("TRN2", target_bir_lowering=False, debug=False,
                   num_devices=N_CORES)

    xt_d = nc.dram_tensor("xt", [HID, S], BF16, kind="ExternalInput")
    wqk_d = nc.dram_tensor("wqk", [HID, 512], BF16, kind="ExternalInput")
    wo_d = nc.dram_tensor("wo", [256, HID], BF16, kind="ExternalInput")
    cosq_d = nc.dram_tensor("cosq", [128, S], F32, kind="ExternalInput")
    sinq_d = nc.dram_tensor("sinq", [128, S], F32, kind="ExternalInput")
    coss_d = nc.dram_tensor("coss", [128, S], F32, kind="ExternalInput")
    sins_d = nc.dram_tensor("sins", [128, S], F32, kind="ExternalInput")
    maskb_d = nc.dram_tensor("maskb", [nmix, 128, 512], F32,
                             kind="ExternalInput")
    out_d = nc.dram_tensor("out", [S, HID], F32, kind="ExternalOutput")
    # pairwise k/v exchange: each core projects q plus ONE of (k, v) for
    # its GQA group ("s", in [hd, tok] layout); the pair AllGather makes
    # both halves visible as cc_out[T] = [k(2x128x512) | v(2x128x512)]
    # identically on both cores (rank order), so the consuming program is
    # parity-independent.
    cc_in = nc.dram_tensor("cc_in", [NT, 2, 128, 512], BF16, kind="Internal")
    cc_out = nc.dram_tensor("cc_out", [NT, 2, 2, 128, 512], BF16,
                            kind="Internal")

    with ExitStack() as ctx:
        tc = ctx.enter_context(TC(nc))
        P = lambda name, bufs, space="SBUF": ctx.enter_context(
            tc.tile_pool(name=name, bufs=bufs, space=space))

        wpool = P("w", 1)
        xpool = P("x", 2)
        cspool = P("cs", 2)
        qkpool = P("qk", 1)
        vpool = P("v", 1)
        spool = P("s", 2)
        vspool = P("vs", 2)
        tmppool = P("tmp", 4)
        upool = P("u", 4)
        epool = P("e", 10)
        mpool = P("m", 3)
        apool = P("a", 1)
        zpool = P("z", 2)
        zapool = P("za", 2)
        opool = P("o", 4)
        rpool = P("r", 1)
        dpool = P("d", 1, "DRAM")

        ps_qk = P("ps_qk", 2, "PSUM")
        ps_v = P("ps_v", 1, "PSUM")  # shared: proj tiles + transpose pads
        ps_s = P("ps_s", 3, "PSUM")
        ps_o = P("ps_o", 2, "PSUM")

        # --- resident weights / constants (wqk streams in chunk-
        # interleaved inside phase A block 0) ---
        wqk = wpool.tile([128, KC * 512], BF16, tag="wqk")
        wo = wpool.tile([128, 2 * HID], BF16, tag="wo")
        nc.sync.dma_start(
            wo[:].rearrange("p (c f) -> p c f", f=HID),
            wo_d[:, :].rearrange("(c p) f -> p c f", p=128))
        ones = wpool.tile([128, 1], BF16, tag="ones")
        nc.gpsimd.memset(ones[:], 1.0)
        ident = wpool.tile([128, 128], BF16, tag="ident")
        make_identity(nc, ident[:])

        # persistent activations (bf16, [128, S] each)
        qlo = qkpool.tile([128, S], BF16, tag="qlo")
        qhi = qkpool.tile([128, S], BF16, tag="qhi")
        klo = qkpool.tile([128, S], BF16, tag="klo")
        khi = qkpool.tile([128, S], BF16, tag="khi")
        vt = vpool.tile([128, (S // 128) * 256], BF16, tag="vt")
        alo = apool.tile([128, S], BF16, tag="alo")
        ahi = apool.tile([128, S], BF16, tag="ahi")
        rc = rpool.tile([128, S // 128], F32, tag="rc")
        zc = rpool.tile([128, S // 128], F32, tag="zc")
        zdram = dpool.tile([NT, 512], F32, tag="zdram")

        def phase_a(T):
            """Return emission units (closures) for QKV block T: project
            q + s (s = k on even cores, v on odd — distinguished purely by
            input data: the wqk second half and the coss/sins tables, which
            are identity for v), then exchange s within the core pair and
            transpose the received v into [tok, hd] layout."""
            c0 = T * 512
            xt = xpool.tile([128, KC * 512], BF16, tag="xt")
            slo = spool.tile([128, 512], BF16, tag="slo")
            shi = spool.tile([128, 512], BF16, tag="shi")
            qk_dst = [(qlo, c0), (qhi, c0), (slo, 0), (shi, 0)]
            units = []

            def dma_unit():
                if T == 0:
                    # chunk-interleaved so the first matmul only waits for
                    # chunk 0, not the whole 4.5 MB of weights+activations
                    for kc in range(KC):
                        nc.sync.dma_start(
                            wqk[:, kc * 512:(kc + 1) * 512],
                            wqk_d[kc * 128:(kc + 1) * 128, :])
                        nc.sync.dma_start(
                            xt[:, kc * 512:(kc + 1) * 512],
                            xt_d[kc * 128:(kc + 1) * 128, c0:c0 + 512])
                else:
                    nc.sync.dma_start(
                        xt[:].rearrange("p (c s) -> p c s", s=512),
                        xt_d[:, c0:c0 + 512].rearrange("(c p) s -> p c s",
                                                       p=128))
            units.append(dma_unit)

            cosq = cspool.tile([128, 512], F32, tag="cosq")
            sinq = cspool.tile([128, 512], F32, tag="sinq")
            coss = cspool.tile([128, 512], F32, tag="coss")
            sins = cspool.tile([128, 512], F32, tag="sins")
            cs_pair = [(cosq, sinq), (coss, sins)]

            def cs_unit():
                nc.sync.dma_start(cosq[:], cosq_d[:, c0:c0 + 512])
                nc.sync.dma_start(sinq[:], sinq_d[:, c0:c0 + 512])
                nc.sync.dma_start(coss[:], coss_d[:, c0:c0 + 512])
                nc.sync.dma_start(sins[:], sins_d[:, c0:c0 + 512])
            units.append(cs_unit)

            def rope_pair(plo, phi, pair):
                cos, sin = cs_pair[pair]
                (dlo, o), (dhi, _) = qk_dst[2 * pair], qk_dst[2 * pair + 1]
                t1 = tmppool.tile([128, 512], F32, tag="tmp")
                nc.vector.tensor_mul(t1[:], phi[:], sin[:])
                t2 = tmppool.tile([128, 512], F32, tag="tmp")
                nc.vector.tensor_mul(t2[:], plo[:], cos[:])
                nc.vector.tensor_sub(dlo[:, o:o + 512], t2[:], t1[:])
                t3 = tmppool.tile([128, 512], F32, tag="tmp")
                nc.vector.tensor_mul(t3[:], plo[:], sin[:])
                t4 = tmppool.tile([128, 512], F32, tag="tmp")
                nc.vector.tensor_mul(t4[:], phi[:], cos[:])
                nc.vector.tensor_add(dhi[:, o:o + 512], t4[:], t3[:])

            pp = {}

            def qk_unit(ft):
                ps = ps_qk.tile([128, 512], F32, tag="ps_qk")
                for kc in range(KC):
                    nc.tensor.matmul(
                        ps[:],
                        wqk[:, kc * 512 + ft * 128: kc * 512 + ft * 128 + 128],
                        xt[:, kc * 512:(kc + 1) * 512],
                        start=(kc == 0), stop=(kc == KC - 1))
                pp[ft] = ps
                if ft % 2 == 1:  # rotate the (lo, hi) pair
                    rope_pair(pp[ft - 1], pp[ft], ft // 2)

            def qk_chunk_major():
                # block 0 is paced by the weight/activation DMAs: keep 4
                # accumulations in flight (borrowing idle B-phase banks) so
                # each arriving chunk feeds 4 matmuls
                psA0 = ps_qk.tile([128, 512], F32, tag="ps_qk")
                psA1 = ps_qk.tile([128, 512], F32, tag="ps_qk")
                psA2 = ps_s.tile([128, 512], F32, tag="ps_s")
                psA3 = ps_o.tile([128, 512], F32, tag="ps_o")
                psA = [psA0, psA1, psA2, psA3]
                for kc in range(KC):
                    for ft in range(4):
                        nc.tensor.matmul(
                            psA[ft][:],
                            wqk[:, kc * 512 + ft * 128: kc * 512 + ft * 128 + 128],
                            xt[:, kc * 512:(kc + 1) * 512],
                            start=(kc == 0), stop=(kc == KC - 1))
                for pair in range(2):
                    rope_pair(psA[2 * pair], psA[2 * pair + 1], pair)

            if T == 0:
                units.append(qk_chunk_major)
            else:
                for ft in range(4):
                    units.append(lambda ft=ft: qk_unit(ft))

            def send_unit():
                nc.sync.dma_start(cc_in[T, 0, :, :], slo[:])
                nc.sync.dma_start(cc_in[T, 1, :, :], shi[:])

            def cc_unit():
                nc.gpsimd.collective_compute(
                    "AllGather", mybir.AluOpType.bypass, CC_GROUPS,
                    ins=[cc_in[T, :, :, :]], outs=[cc_out[T, :, :, :, :]])

            vsl = vspool.tile([128, 512], BF16, tag="vsl")
            vsh = vspool.tile([128, 512], BF16, tag="vsh")

            def recv_unit():
                nc.sync.dma_start(klo[:, c0:c0 + 512], cc_out[T, 0, 0, :, :])
                nc.sync.dma_start(khi[:, c0:c0 + 512], cc_out[T, 0, 1, :, :])
                nc.sync.dma_start(vsl[:], cc_out[T, 1, 0, :, :])
                nc.sync.dma_start(vsh[:], cc_out[T, 1, 1, :, :])

            def tr_unit(sub):
                tok = T * 4 + sub
                tp = ps_v.tile([128, 256], BF16, tag="ps_v")
                nc.tensor.transpose(tp[:, 0:128],
                                    vsl[:, sub * 128:(sub + 1) * 128],
                                    ident[:])
                nc.tensor.transpose(tp[:, 128:256],
                                    vsh[:, sub * 128:(sub + 1) * 128],
                                    ident[:])
                nc.vector.tensor_copy(vt[:, tok * 256:(tok + 1) * 256], tp[:])

            units.append(send_unit)
            units.append(cc_unit)
            consume = [recv_unit] + [lambda sub=sub: tr_unit(sub)
                                     for sub in range(4)]
            return units, consume

        def phase_b(qb):
            """Return emission units for attention q-block qb, one per
            k-chunk. The S matmuls of j lead the E-consumers of j-1 so the
            ACT chain has a full PE iteration of slack."""
            c0 = qb * 512
            zacc = zapool.tile([128, 512], F32, tag="za")
            olo = ps_o.tile([128, 512], F32, tag="ps_o")
            ohi = ps_o.tile([128, 512], F32, tag="ps_o")
            row = plan[qb]
            state = {}

            def s_unit(idx):
                j, mix, q0, q1 = row[idx]
                w = q1 - q0
                if mix >= 0:
                    mk = mpool.tile([128, 512], F32, tag="m")
                    nc.sync.dma_start(mk[:, :w], maskb_d[mix, :, q0:q1])
                else:
                    mk = None
                sps = ps_s.tile([128, 512], F32, tag="ps_s")
                nc.tensor.matmul(sps[:, q0:q1], klo[:, j * 128:(j + 1) * 128],
                                 qlo[:, c0 + q0:c0 + q1],
                                 start=True, stop=False)
                nc.tensor.matmul(sps[:, q0:q1], khi[:, j * 128:(j + 1) * 128],
                                 qhi[:, c0 + q0:c0 + q1],
                                 start=False, stop=True)
                e = epool.tile([128, 512], BF16, tag="e")
                u = upool.tile([128, 512], F32, tag="u")
                nc.scalar.activation(u[:, :w], sps[:, q0:q1], AF.Tanh,
                                     scale=1.0 / SOFTCAP)
                if mk is not None:
                    u2 = upool.tile([128, 512], F32, tag="u")
                    nc.vector.tensor_add(u2[:, :w], u[:, :w], mk[:, :w])
                    u = u2
                nc.scalar.activation(e[:, q0:q1], u[:, :w], AF.Exp,
                                     scale=SOFTCAP)
                if idx == 0:
                    nc.vector.tensor_copy(zacc[:], e[:])
                else:
                    nc.vector.tensor_add(zacc[:, q0:q1], zacc[:, q0:q1],
                                         e[:, q0:q1])
                state[idx] = e

            def mm_unit(idx):
                j, _, q0, q1 = row[idx]
                e = state.pop(idx)
                first, last = idx == 0, idx == len(row) - 1
                nc.tensor.matmul(olo[:, q0:q1], vt[:, j * 256:j * 256 + 128],
                                 e[:, q0:q1], start=first, stop=last)
                nc.tensor.matmul(ohi[:, q0:q1],
                                 vt[:, j * 256 + 128:(j + 1) * 256],
                                 e[:, q0:q1], start=first, stop=last)

            def tail_unit():
                nc.vector.tensor_copy(alo[:, c0:c0 + 512], olo[:])
                nc.vector.tensor_copy(ahi[:, c0:c0 + 512], ohi[:])
                # single f32->bf16 rounding of zacc so the 128-partition
                # reduction runs as a 1-cycle/row bf16 matmul (fp32 is 4x)
                zaccb = zpool.tile([128, 512], BF16, tag="zb")
                nc.scalar.copy(zaccb[:], zacc[:])
                zps = ps_s.tile([1, 512], F32, tag="ps_s")
                nc.tensor.matmul(zps[:], ones[:], zaccb[:],
                                 start=True, stop=True)
                zrow = zpool.tile([1, 512], F32, tag="z")
                nc.vector.tensor_copy(zrow[:], zps[:])
                nc.sync.dma_start(zdram[qb, :], zrow[:])
                # incremental 1/Z so proj for this q-block can start now
                nc.sync.dma_start(
                    zc[:, 4 * qb:4 * qb + 4],
                    zdram[qb, :].rearrange("(b p) -> p b", p=128))
                nc.vector.reciprocal(rc[:, 4 * qb:4 * qb + 4],
                                     zc[:, 4 * qb:4 * qb + 4])

            units = [lambda: s_unit(0)]
            for idx in range(1, len(row)):
                units.append(lambda idx=idx: (s_unit(idx), mm_unit(idx - 1)))
            units.append(lambda: (mm_unit(len(row) - 1), tail_unit()))
            return units

        # PE warmup: a few throwaway matmuls so HAM reaches 8/8 before
        # the first real accumulation
        scratch = wpool.tile([128, 512], BF16, tag="scratch")
        nc.gpsimd.memset(scratch[:], 0.0)
        wps = ps_s.tile([128, 512], F32, tag="ps_s")
        for _ in range(12):
            nc.tensor.matmul(wps[:], scratch[:, :128], scratch[:],
                             start=True, stop=True)

        # output projection units (one per (tok-tile, feat-block)); the
        # 1/Z normalization is fused into the PSUM->SBUF copy. These are
        # woven into later B phases so the 37 MB output DMA spreads over
        # the whole kernel instead of saturating the tail.
        fbs = [(0, 512), (512, 512), (1024, 512), (1536, 512), (2048, 256)]

        def proj_unit(t, fi):
            f0, fw = fbs[fi]
            pool = ps_qk if fi % 3 < 2 else ps_v
            ps = pool.tile([128, 512], F32, tag=pool.name)
            nc.tensor.matmul(ps[:, :fw], alo[:, t * 128:(t + 1) * 128],
                             wo[:, f0:f0 + fw], start=True, stop=False)
            nc.tensor.matmul(ps[:, :fw], ahi[:, t * 128:(t + 1) * 128],
                             wo[:, HID + f0:HID + f0 + fw],
                             start=False, stop=True)
            osb = opool.tile([128, 512], F32, tag="o")
            if fi % 2 == 0:
                nc.scalar.activation(osb[:, :fw], ps[:, :fw], AF.Copy,
                                     scale=rc[:, t:t + 1])
            else:
                nc.vector.tensor_scalar_mul(osb[:, :fw], ps[:, :fw],
                                            rc[:, t:t + 1])
            nc.sync.dma_start(out_d[t * 128:(t + 1) * 128, f0:f0 + fw],
                              osb[:, :fw])

        def phase_c(qb):
            return [lambda t=t, fi=fi: proj_unit(t, fi)
                    for t in range(4 * qb, 4 * qb + 4)
                    for fi in range(len(fbs))]

        def weave(bunits, aunits):
            """Alternate B and A units so stalled B consumers never block
            independent A matmuls in the in-order PE queue."""
            out = []
            na, nb = len(aunits), len(bunits)
            ai = 0
            for bi, bu in enumerate(bunits):
                out.append(bu)
                want = (bi + 1) * na // nb
                while ai < want:
                    out.append(aunits[ai])
                    ai += 1
            out.extend(aunits[ai:])
            return out

        # A runs two blocks ahead of B; each block's exchange-consume units
        # (recv DMAs + v transposes, the first PE-queue instructions that
        # WAIT on the collective) are woven a full B-phase after the
        # collective was issued, so the pair AllGather never stalls the
        # in-order PE queue.
        consumes = {}
        prod0, consumes[0] = phase_a(0)
        prod1, consumes[1] = phase_a(1)
        for u in prod0 + prod1 + consumes.pop(0):
            u()
        for T in range(NT):
            bunits = phase_b(T)
            aunits = []
            if T + 2 < NT:
                prod, consumes[T + 2] = phase_a(T + 2)
                aunits += prod
            if T >= 1:
                aunits += phase_c(T - 1)
            if T + 1 in consumes:
                aunits += consumes.pop(T + 1)
            with nc.named_scope(f"B{T}"):
                for u in weave(bunits, aunits):
                    u()
        with nc.named_scope("Ctail"):
            for qb in (NT - 1,):
                for u in phase_c(qb):
                    u()


    split_multi_waits(nc)
    return nc


def kernel(hidden_states, attention_mask, position_ids, Wqkv, Wo):
    bf16 = ml_dtypes.bfloat16
    hidden = np.asarray(hidden_states, np.float32)
    S = hidden.shape[1]
    X = hidden[0]  # [S, HID]
    XT = np.ascontiguousarray(X.T).astype(bf16)  # [HID, S]

    pos = np.asarray(position_ids)[0].astype(np.float64)
    inv = 1.0 / (ROPE_THETA ** (np.arange(0, HD, 2, dtype=np.float64) / HD))
    freqs = inv[:, None] * pos[None, :]  # [128, S]
    cosT = np.cos(freqs).astype(np.float32)
    sinT = np.sin(freqs).astype(np.float32)

    plan, maskb = _classify_mask(attention_mask, S)

    Wqkv = np.asarray(Wqkv, np.float32)
    Wo = np.asarray(Wo, np.float32)

    one_cs = np.ones_like(cosT)
    zero_cs = np.zeros_like(sinT)

    in_maps = []
    for c in range(N_CORES):
        g = c // (NH // NKV)
        wq = Wqkv[c * HD:(c + 1) * HD] * SCALE  # exact: SCALE = 2**-4
        wk = Wqkv[NH * HD + g * HD: NH * HD + (g + 1) * HD]
        wv = Wqkv[(NH + NKV) * HD + g * HD: (NH + NKV) * HD + (g + 1) * HD]
        # even core of a pair projects+sends k (RoPE'd), odd projects+sends
        # v (coss=1/sins=0 makes the s-RoPE the identity)
        ws = wk if c % 2 == 0 else wv
        wqk = np.ascontiguousarray(
            np.concatenate([wq.T, ws.T], axis=1)).astype(bf16)
        wot = np.ascontiguousarray(Wo[:, c * HD:(c + 1) * HD].T).astype(bf16)
        in_maps.append({
            "xt": XT, "wqk": wqk, "wo": wot,
            "cosq": cosT, "sinq": sinT,
            "coss": cosT if c % 2 == 0 else one_cs,
            "sins": sinT if c % 2 == 0 else zero_cs,
            "maskb": maskb,
        })

    nc = _build(S, plan, maskb.shape[0])
    res = run_bass_kernel_spmd(nc, in_maps, list(range(N_CORES)),
                               trace=TRACE)
    out = res.results[0]["out"].astype(np.float64)
    for c in range(1, N_CORES):
        out += res.results[c]["out"]
    kernel.last_exec_time_ns = res.exec_time_ns
    kernel.last_results = res
    return out[None].astype(np.float32)


kernel.last_exec_time_ns = None
kernel.last_results = None



# revision 43
# speedup vs baseline: 1.2775x; 1.0446x over previous
"""Gemma2 fused attention (B=1, S=4096, HID=2304, NH=8, NKV=4, HD=256,
sliding window 2048, softcap 50) on 8 Trainium2 NeuronCores.

Sharding: one query head per core (tensor parallel). Each GQA pair of
cores splits its shared k/v projection: the even core projects+RoPEs k,
the odd core projects v (identity "RoPE" via cos=1/sin=0 tables, so the
program stays SPMD-identical and only input data differs), then a
pairwise AllGather publishes [k | v] identically on both cores. The
received v ([hd, tok]) is PE-transposed into the [tok, hd] layout the
attnT matmuls need. o_proj is sharded over the contraction dim; bf16
per-core partials are summed on the host.

Per-core math (core c, head h=c, kv group g=c//2):
  qT,sT = (W @ X.T) in [head_dim, tok] layout, RoPE'd on device (bf16
  cos/sin tables from host; attention scale folded into Wq exactly).
  S.T[k,q] = kT.T @ qT; u = tanh(S.T/50); E = exp(50*u + mask) in bf16
  (softcap bounds logits to +-50 so no max-subtraction is needed).
  Mask handled per 128(k) x 512(q) block: all-zero blocks skip the add,
  fully-masked blocks are skipped entirely, mixed blocks add mask*0.02
  from a host-packed bf16 block stack, and fully-masked q-columns are
  trimmed from the matmuls/activations (data-driven, no pattern
  assumption).
  Z = ones.T @ E(bf16) as a PSUM row; attnT = (E @ v).T via lhsT=v.
  out_partial[tok, 2304] = attnT.T @ WoT with 1/Z fused into the
  PSUM->SBUF drain. Host unshuffles the block layout and sums partials.

Schedule: A (proj+exchange) runs two 512-token blocks ahead of B
(attention); exchange-consume units (recv + v-transpose, the first
PE-queue ops that wait on the collective) are woven a full B-phase
after their collective was issued; proj(T-1) units are woven into B_T
with the PSUM drain copy lagging one unit behind its matmuls; xt loads
are host-preshuffled block-major so each block is one contiguous DMA.
"""

import numpy as np
import ml_dtypes
from contextlib import ExitStack

import concourse.bass as bass
import concourse.tile as tile
import concourse.mybir as mybir
from concourse.bass_utils import run_bass_kernel_spmd
from concourse.masks import make_identity
from concourse.vector_clock import ScopedClock

N_CORES = 8
HID = 2304
NH, NKV, HD = 8, 4, 256
SCALE = 256.0 ** -0.5
SOFTCAP = 50.0
ROPE_THETA = 10000.0
KC = HID // 128  # 18 contraction chunks for the projections
CC_GROUPS = [[0, 1], [2, 3], [4, 5], [6, 7]]  # GQA pair exchange

BF16 = mybir.dt.bfloat16
F32 = mybir.dt.float32
AF = mybir.ActivationFunctionType

TRACE = False  # test harness flips this to get NTFF exec time


class TC(tile.TileContext):
    """TileContext whose final drain splits sem waits one-per-instruction
    (this walrus rejects instructions carrying more than one wait)."""

    def _drain_and_barrier(self, tick_clock, wait_clock):
        probe = self.nc.sync.nop(nofuse=True, hint="drain_waits")
        wait_clock.add_sem_waits(
            probe.ins, ScopedClock({None: tick_clock.global_clock})
        )
        waits = list(probe.ins.sync_info.on_wait)
        probe.ins.sync_info.on_wait = waits[:1]
        rest = waits[1:]
        while rest:
            extra = self.nc.sync.nop(nofuse=True, hint="drain_waits")
            extra.ins.sync_info = mybir.SyncInfo(on_wait=rest[:1], on_update=[])
            rest = rest[1:]
        self.nc.sync.drain()
        self.nc.all_engine_barrier()
        popped = self.nc._tile_sem_poison_stack.pop()
        assert popped is self._sem_poison
        self.nc.clear_and_free_semaphores(list(self.sems.allocated().values()))
        self.nc.all_engine_barrier()


def split_multi_waits(nc):
    """Split multi-wait instructions: extras move onto same-engine NoOps
    inserted immediately before (engines execute in program order)."""
    ctr = 0
    for f in nc.m.functions:
        for b in f.blocks:
            insts = list(b.instructions)
            new = []
            changed = False
            for inst in insts:
                si = inst.sync_info
                if si is not None and len(si.on_wait) > 1:
                    waits = list(si.on_wait)
                    for w in waits[:-1]:
                        ctr += 1
                        nop = mybir.InstNoOp(
                            name=f"I-waitsplit-{ctr}",
                            engine=inst.engine,
                            debug=inst.debug,
                            sync_info=mybir.SyncInfo(on_wait=[w], on_update=[]),
                        )
                        new.append(nop)
                    inst.sync_info = mybir.SyncInfo(
                        on_wait=[waits[-1]], on_update=list(si.on_update)
                    )
                    changed = True
                new.append(inst)
            if changed:
                b.instructions = new
    return ctr


def _classify_mask(mask, S):
    """Per (k-chunk 128, q-block 512) block: 'skip' (fully masked),
    'clean' (all zero) or mixed (apply additively). Each plan entry is
    (j, mix, q0, q1): only q-columns [q0, q1) have any unmasked k in the
    chunk, so S/E/attnT work is restricted to that slice. The first entry
    of every row is full-width so it can init the PSUM accumulation and
    zacc. Returns plan and the packed mixed-block stack (already scaled
    by 1/SOFTCAP)."""
    maskT = np.ascontiguousarray(np.asarray(mask, np.float32)[0, 0].T)  # [k, q]
    nj, nq = S // 128, S // 512
    blocks = maskT.reshape(nj, 128, nq, 512)
    mx = blocks.max(axis=(1, 3))
    mn = blocks.min(axis=(1, 3))
    skip = mx < -1e8
    clean = (mx == 0.0) & (mn == 0.0)
    plan = []
    mix_blocks = []
    for qb in range(nq):
        row = []
        for j in range(nj):
            if skip[j, qb]:
                continue
            if clean[j, qb]:
                row.append((j, -1, 0, 512))
            else:
                blk = maskT[j * 128:(j + 1) * 128, qb * 512:(qb + 1) * 512]
                col_ok = blk.max(axis=0) > -1e8
                q0 = int(col_ok.argmax())
                q1 = 512 - int(col_ok[::-1].argmax())
                if not col_ok[q0:q1].all():
                    q0, q1 = 0, 512  # non-contiguous valid span: no trim
                mix_blocks.append((blk * (1.0 / SOFTCAP)).astype(np.float32))
                row.append((j, len(mix_blocks) - 1, q0, q1))
        if not row:
            # fully-masked q-block (unreachable for causal masks): keep the
            # diagonal chunks so the PSUM accumulations are still defined
            for j in range(4 * qb, 4 * qb + 4):
                mix_blocks.append(
                    (maskT[j * 128:(j + 1) * 128, qb * 512:(qb + 1) * 512]
                     * (1.0 / SOFTCAP)).astype(np.float32))
                row.append((j, len(mix_blocks) - 1, 0, 512))
        # first entry must be full-width (inits PSUM + zacc). Prefer the
        # lowest-j full-width entry (oldest k/v data, longest ready) so the
        # high-j chunks — which depend on the freshest k/v exchange — run
        # last; else widen the first (safe for mixed entries: the additive
        # mask zeroes E outside the valid span).
        full = next((i for i, e in enumerate(row)
                     if e[2] == 0 and e[3] == 512), None)
        if full is None:
            j, mix, _, _ = row[0]
            assert mix >= 0
            row[0] = (j, mix, 0, 512)
        else:
            row = [row[full]] + row[:full] + row[full + 1:]
        plan.append(row)
    if mix_blocks:
        maskb = np.stack(mix_blocks)
    else:
        maskb = np.zeros((1, 128, 512), np.float32)
    return plan, maskb


def _build(S, plan, nmix):
    """Emit the SPMD program (identical for all cores; only data differs)."""
    NT = S // 512  # token/query 512-blocks
    nc = bass.Bass("TRN2", target_bir_lowering=False, debug=False,
                   num_devices=N_CORES)

    # xt is host-preshuffled block-major: xt_d[p, (T, kc, s)] =
    # X.T[kc*128 + p, T*512 + s], so each block's 2.25 MB loads as one
    # DMA of 128 contiguous 18 KB partition runs instead of 2304 1 KB
    # descriptors.
    NTb = S // 512
    xt_d = nc.dram_tensor("xt", [128, NTb * KC * 512], BF16,
                          kind="ExternalInput")
    wqk_d = nc.dram_tensor("wqk", [HID, 512], BF16, kind="ExternalInput")
    wo_d = nc.dram_tensor("wo", [256, HID], BF16, kind="ExternalInput")
    cosq_d = nc.dram_tensor("cosq", [128, S], F32, kind="ExternalInput")
    sinq_d = nc.dram_tensor("sinq", [128, S], F32, kind="ExternalInput")
    coss_d = nc.dram_tensor("coss", [128, S], F32, kind="ExternalInput")
    sins_d = nc.dram_tensor("sins", [128, S], F32, kind="ExternalInput")
    maskb_d = nc.dram_tensor("maskb", [nmix, 128, 512], F32,
                             kind="ExternalInput")
    # block-layout output (one fully-contiguous write per proj unit); the
    # host unshuffles back to [S, HID]. bf16 partials: each core's partial
    # is rounded once (~0.2% rms), the host sums in f32.
    out_d = nc.dram_tensor("out", [S // 128, 5, 128, 512], BF16,
                           kind="ExternalOutput")
    # pairwise k/v exchange: each core projects q plus ONE of (k, v) for
    # its GQA group ("s", in [hd, tok] layout); the pair AllGather makes
    # both halves visible as cc_out[T] = [k(2x128x512) | v(2x128x512)]
    # identically on both cores (rank order), so the consuming program is
    # parity-independent.
    cc_in = nc.dram_tensor("cc_in", [NT, 2, 128, 512], BF16, kind="Internal")
    cc_out = nc.dram_tensor("cc_out", [NT, 2, 2, 128, 512], BF16,
                            kind="Internal")

    with ExitStack() as ctx:
        tc = ctx.enter_context(TC(nc))
        P = lambda name, bufs, space="SBUF": ctx.enter_context(
            tc.tile_pool(name=name, bufs=bufs, space=space))

        wpool = P("w", 1)
        xpool = P("x", 3)
        cspool = P("cs", 2)
        qkpool = P("qk", 1)
        vpool = P("v", 1)
        spool = P("s", 2)
        vspool = P("vs", 2)
        tmppool = P("tmp", 4)
        upool = P("u", 4)
        epool = P("e", 10)
        mpool = P("m", 3)
        apool = P("a", 1)
        zpool = P("z", 2)
        zapool = P("za", 2)
        opool = P("o", 4)
        rpool = P("r", 1)
        dpool = P("d", 1, "DRAM")

        ps_qk = P("ps_qk", 3, "PSUM")
        ps_v = P("ps_v", 1, "PSUM")  # shared: proj tiles + transpose pads
        ps_s = P("ps_s", 2, "PSUM")
        ps_o = P("ps_o", 2, "PSUM")

        # --- resident weights / constants (wqk streams in chunk-
        # interleaved inside phase A block 0) ---
        wqk = wpool.tile([128, KC * 512], BF16, tag="wqk")
        wo = wpool.tile([128, 2 * HID], BF16, tag="wo")
        nc.sync.dma_start(
            wo[:].rearrange("p (c f) -> p c f", f=HID),
            wo_d[:, :].rearrange("(c p) f -> p c f", p=128))
        ones = wpool.tile([128, 1], BF16, tag="ones")
        nc.gpsimd.memset(ones[:], 1.0)
        ident = wpool.tile([128, 128], BF16, tag="ident")
        make_identity(nc, ident[:])

        # persistent activations (bf16, [128, S] each)
        qlo = qkpool.tile([128, S], BF16, tag="qlo")
        qhi = qkpool.tile([128, S], BF16, tag="qhi")
        klo = qkpool.tile([128, S], BF16, tag="klo")
        khi = qkpool.tile([128, S], BF16, tag="khi")
        vt = vpool.tile([128, (S // 128) * 256], BF16, tag="vt")
        alo = apool.tile([128, S], BF16, tag="alo")
        ahi = apool.tile([128, S], BF16, tag="ahi")
        rc = rpool.tile([128, S // 128], F32, tag="rc")
        zc = rpool.tile([128, S // 128], F32, tag="zc")
        zdram = dpool.tile([NT, 512], F32, tag="zdram")

        def phase_a(T):
            """Return emission units (closures) for QKV block T: project
            q + s (s = k on even cores, v on odd — distinguished purely by
            input data: the wqk second half and the coss/sins tables, which
            are identity for v), then exchange s within the core pair and
            transpose the received v into [tok, hd] layout."""
            c0 = T * 512
            xt = xpool.tile([128, KC * 512], BF16, tag="xt")
            slo = spool.tile([128, 512], BF16, tag="slo")
            shi = spool.tile([128, 512], BF16, tag="shi")
            qk_dst = [(qlo, c0), (qhi, c0), (slo, 0), (shi, 0)]
            units = []

            xb0 = T * KC * 512

            def dma_unit():
                if T == 0:
                    # chunk-interleaved so the first matmul only waits for
                    # chunk 0, not the whole 4.5 MB of weights+activations
                    for kc in range(KC):
                        nc.sync.dma_start(
                            wqk[:, kc * 512:(kc + 1) * 512],
                            wqk_d[kc * 128:(kc + 1) * 128, :])
                        nc.sync.dma_start(
                            xt[:, kc * 512:(kc + 1) * 512],
                            xt_d[:, xb0 + kc * 512: xb0 + (kc + 1) * 512])
                else:
                    nc.sync.dma_start(
                        xt[:], xt_d[:, xb0:xb0 + KC * 512])
            units.append(dma_unit)

            cosq = cspool.tile([128, 512], F32, tag="cosq")
            sinq = cspool.tile([128, 512], F32, tag="sinq")
            coss = cspool.tile([128, 512], F32, tag="coss")
            sins = cspool.tile([128, 512], F32, tag="sins")
            cs_pair = [(cosq, sinq), (coss, sins)]

            def cs_unit():
                nc.sync.dma_start(cosq[:], cosq_d[:, c0:c0 + 512])
                nc.sync.dma_start(sinq[:], sinq_d[:, c0:c0 + 512])
                nc.sync.dma_start(coss[:], coss_d[:, c0:c0 + 512])
                nc.sync.dma_start(sins[:], sins_d[:, c0:c0 + 512])
            units.append(cs_unit)

            def rope_pair(plo, phi, pair):
                cos, sin = cs_pair[pair]
                (dlo, o), (dhi, _) = qk_dst[2 * pair], qk_dst[2 * pair + 1]
                t1 = tmppool.tile([128, 512], F32, tag="tmp")
                nc.vector.tensor_mul(t1[:], phi[:], sin[:])
                t2 = tmppool.tile([128, 512], F32, tag="tmp")
                nc.vector.tensor_mul(t2[:], plo[:], cos[:])
                nc.vector.tensor_sub(dlo[:, o:o + 512], t2[:], t1[:])
                t3 = tmppool.tile([128, 512], F32, tag="tmp")
                nc.vector.tensor_mul(t3[:], plo[:], sin[:])
                t4 = tmppool.tile([128, 512], F32, tag="tmp")
                nc.vector.tensor_mul(t4[:], phi[:], cos[:])
                nc.vector.tensor_add(dhi[:, o:o + 512], t4[:], t3[:])

            pp = {}

            def qk_unit(ft):
                ps = ps_qk.tile([128, 512], F32, tag="ps_qk")
                for kc in range(KC):
                    nc.tensor.matmul(
                        ps[:],
                        wqk[:, kc * 512 + ft * 128: kc * 512 + ft * 128 + 128],
                        xt[:, kc * 512:(kc + 1) * 512],
                        start=(kc == 0), stop=(kc == KC - 1))
                pp[ft] = ps
                if ft % 2 == 1:  # rotate the (lo, hi) pair
                    rope_pair(pp[ft - 1], pp[ft], ft // 2)

            def qk_chunk_major():
                # block 0 is paced by the weight/activation DMAs: keep 4
                # accumulations in flight (borrowing idle B-phase banks) so
                # each arriving chunk feeds 4 matmuls
                psA0 = ps_qk.tile([128, 512], F32, tag="ps_qk")
                psA1 = ps_qk.tile([128, 512], F32, tag="ps_qk")
                psA2 = ps_s.tile([128, 512], F32, tag="ps_s")
                psA3 = ps_o.tile([128, 512], F32, tag="ps_o")
                psA = [psA0, psA1, psA2, psA3]
                for kc in range(KC):
                    for ft in range(4):
                        nc.tensor.matmul(
                            psA[ft][:],
                            wqk[:, kc * 512 + ft * 128: kc * 512 + ft * 128 + 128],
                            xt[:, kc * 512:(kc + 1) * 512],
                            start=(kc == 0), stop=(kc == KC - 1))
                for pair in range(2):
                    rope_pair(psA[2 * pair], psA[2 * pair + 1], pair)

            if T == 0:
                units.append(qk_chunk_major)
            else:
                for ft in range(4):
                    units.append(lambda ft=ft: qk_unit(ft))

            def send_unit():
                nc.sync.dma_start(cc_in[T, 0, :, :], slo[:])
                nc.sync.dma_start(cc_in[T, 1, :, :], shi[:])

            def cc_unit():
                nc.gpsimd.collective_compute(
                    "AllGather", mybir.AluOpType.bypass, CC_GROUPS,
                    ins=[cc_in[T, :, :, :]], outs=[cc_out[T, :, :, :, :]])

            vsl = vspool.tile([128, 512], BF16, tag="vsl")
            vsh = vspool.tile([128, 512], BF16, tag="vsh")

            def recv_unit():
                nc.sync.dma_start(klo[:, c0:c0 + 512], cc_out[T, 0, 0, :, :])
                nc.sync.dma_start(khi[:, c0:c0 + 512], cc_out[T, 0, 1, :, :])
                nc.sync.dma_start(vsl[:], cc_out[T, 1, 0, :, :])
                nc.sync.dma_start(vsh[:], cc_out[T, 1, 1, :, :])

            def tr_unit(sub):
                tok = T * 4 + sub
                tp = ps_v.tile([128, 256], BF16, tag="ps_v")
                nc.tensor.transpose(tp[:, 0:128],
                                    vsl[:, sub * 128:(sub + 1) * 128],
                                    ident[:])
                nc.tensor.transpose(tp[:, 128:256],
                                    vsh[:, sub * 128:(sub + 1) * 128],
                                    ident[:])
                nc.vector.tensor_copy(vt[:, tok * 256:(tok + 1) * 256], tp[:])

            units.append(send_unit)
            units.append(cc_unit)
            consume = [recv_unit] + [lambda sub=sub: tr_unit(sub)
                                     for sub in range(4)]
            return units, consume

        def phase_b(qb):
            """Return emission units for attention q-block qb, one per
            k-chunk. The S matmuls of j lead the E-consumers of j-1 so the
            ACT chain has a full PE iteration of slack."""
            c0 = qb * 512
            zacc = zapool.tile([128, 512], F32, tag="za")
            olo = ps_o.tile([128, 512], F32, tag="ps_o")
            ohi = ps_o.tile([128, 512], F32, tag="ps_o")
            row = plan[qb]
            state = {}

            def s_unit(idx):
                j, mix, q0, q1 = row[idx]
                w = q1 - q0
                if mix >= 0:
                    mk = mpool.tile([128, 512], F32, tag="m")
                    nc.sync.dma_start(mk[:, :w], maskb_d[mix, :, q0:q1])
                else:
                    mk = None
                sps = ps_s.tile([128, 512], F32, tag="ps_s")
                nc.tensor.matmul(sps[:, q0:q1], klo[:, j * 128:(j + 1) * 128],
                                 qlo[:, c0 + q0:c0 + q1],
                                 start=True, stop=False)
                nc.tensor.matmul(sps[:, q0:q1], khi[:, j * 128:(j + 1) * 128],
                                 qhi[:, c0 + q0:c0 + q1],
                                 start=False, stop=True)
                e = epool.tile([128, 512], BF16, tag="e")
                u = upool.tile([128, 512], F32, tag="u")
                nc.scalar.activation(u[:, :w], sps[:, q0:q1], AF.Tanh,
                                     scale=1.0 / SOFTCAP)
                if mk is not None:
                    u2 = upool.tile([128, 512], F32, tag="u")
                    nc.vector.tensor_add(u2[:, :w], u[:, :w], mk[:, :w])
                    u = u2
                nc.scalar.activation(e[:, q0:q1], u[:, :w], AF.Exp,
                                     scale=SOFTCAP)
                if idx == 0:
                    nc.vector.tensor_copy(zacc[:], e[:])
                else:
                    nc.vector.tensor_add(zacc[:, q0:q1], zacc[:, q0:q1],
                                         e[:, q0:q1])
                state[idx] = e

            def mm_unit(idx):
                j, _, q0, q1 = row[idx]
                e = state.pop(idx)
                first, last = idx == 0, idx == len(row) - 1
                nc.tensor.matmul(olo[:, q0:q1], vt[:, j * 256:j * 256 + 128],
                                 e[:, q0:q1], start=first, stop=last)
                nc.tensor.matmul(ohi[:, q0:q1],
                                 vt[:, j * 256 + 128:(j + 1) * 256],
                                 e[:, q0:q1], start=first, stop=last)

            def tail_unit():
                nc.vector.tensor_copy(alo[:, c0:c0 + 512], olo[:])
                nc.vector.tensor_copy(ahi[:, c0:c0 + 512], ohi[:])
                # single f32->bf16 rounding of zacc so the 128-partition
                # reduction runs as a 1-cycle/row bf16 matmul (fp32 is 4x)
                zaccb = zpool.tile([128, 512], BF16, tag="zb")
                nc.scalar.copy(zaccb[:], zacc[:])
                zps = ps_s.tile([1, 512], F32, tag="ps_s")
                nc.tensor.matmul(zps[:], ones[:], zaccb[:],
                                 start=True, stop=True)
                zrow = zpool.tile([1, 512], F32, tag="z")
                nc.vector.tensor_copy(zrow[:], zps[:])
                nc.sync.dma_start(zdram[qb, :], zrow[:])
                # incremental 1/Z so proj for this q-block can start now
                nc.sync.dma_start(
                    zc[:, 4 * qb:4 * qb + 4],
                    zdram[qb, :].rearrange("(b p) -> p b", p=128))
                nc.vector.reciprocal(rc[:, 4 * qb:4 * qb + 4],
                                     zc[:, 4 * qb:4 * qb + 4])

            units = [lambda: s_unit(0)]
            for idx in range(1, len(row)):
                units.append(lambda idx=idx: (s_unit(idx), mm_unit(idx - 1)))
            units.append(lambda: (mm_unit(len(row) - 1), tail_unit()))
            return units

        # PE warmup: a few throwaway matmuls so HAM reaches 8/8 before
        # the first real accumulation
        scratch = wpool.tile([128, 512], BF16, tag="scratch")
        nc.vector.memset(scratch[:], 0.0)
        wps = ps_s.tile([128, 512], F32, tag="ps_s")
        for _ in range(12):
            nc.tensor.matmul(wps[:], scratch[:, :128], scratch[:],
                             start=True, stop=True)

        # output projection units (one per (tok-tile, feat-block)); the
        # 1/Z normalization is fused into the PSUM->SBUF copy. These are
        # woven into later B phases so the 37 MB output DMA spreads over
        # the whole kernel instead of saturating the tail.
        fbs = [(0, 512), (512, 512), (1024, 512), (1536, 512), (2048, 256)]

        def proj_mm(t, fi, st):
            f0, fw = fbs[fi]
            pool = ps_qk if fi % 3 < 2 else ps_v
            ps = pool.tile([128, 512], F32, tag=pool.name)
            nc.tensor.matmul(ps[:, :fw], alo[:, t * 128:(t + 1) * 128],
                             wo[:, f0:f0 + fw], start=True, stop=False)
            nc.tensor.matmul(ps[:, :fw], ahi[:, t * 128:(t + 1) * 128],
                             wo[:, HID + f0:HID + f0 + fw],
                             start=False, stop=True)
            st[(t, fi)] = ps

        def proj_cp(t, fi, st):
            f0, fw = fbs[fi]
            ps = st.pop((t, fi))
            osb = opool.tile([128, 512], BF16, tag="o")
            if fi % 2 == 0:
                nc.scalar.activation(osb[:, :fw], ps[:, :fw], AF.Copy,
                                     scale=rc[:, t:t + 1])
            else:
                nc.vector.tensor_scalar_mul(osb[:, :fw], ps[:, :fw],
                                            rc[:, t:t + 1])
            nc.sync.dma_start(out_d[t, fi, :, :fw], osb[:, :fw])

        def phase_c(qb):
            # drain copy of proj i-1 rides with the matmuls of proj i, so
            # each copy has a full weave-slot of engine-queue slack before
            # its PSUM bank is recycled
            prs = [(t, fi) for t in range(4 * qb, 4 * qb + 4)
                   for fi in range(len(fbs))]
            st = {}

            def unit(i):
                if i > 0:
                    proj_cp(*prs[i - 1], st)
                proj_mm(*prs[i], st)

            units = [lambda i=i: unit(i) for i in range(len(prs))]
            units.append(lambda: proj_cp(*prs[-1], st))
            return units

        def weave(bunits, aunits):
            """Alternate B and A units so stalled B consumers never block
            independent A matmuls in the in-order PE queue."""
            out = []
            na, nb = len(aunits), len(bunits)
            ai = 0
            for bi, bu in enumerate(bunits):
                out.append(bu)
                want = (bi + 1) * na // nb
                while ai < want:
                    out.append(aunits[ai])
                    ai += 1
            out.extend(aunits[ai:])
            return out

        # A runs two blocks ahead of B; each block's exchange-consume units
        # (recv DMAs + v transposes, the first PE-queue instructions that
        # WAIT on the collective) are woven a full B-phase after the
        # collective was issued, so the pair AllGather never stalls the
        # in-order PE queue.
        consumes = {}
        prod0, consumes[0] = phase_a(0)
        prod1, consumes[1] = phase_a(1)
        # prologue order: both xt DMAs first, A0 compute, then A0's
        # send+collective immediately (so cc_0 runs during A1 compute),
        # then A1, then consume(0)
        for u in ([prod0[0], prod0[1], prod1[0]] + prod0[2:]
                  + [prod1[1]] + prod1[2:] + consumes.pop(0)):
            u()
        for T in range(NT):
            bunits = phase_b(T)
            aunits = []
            if T + 2 < NT:
                prod, consumes[T + 2] = phase_a(T + 2)
                aunits += prod
            if T >= 1:
                aunits += phase_c(T - 1)
            if T + 1 in consumes:
                aunits += consumes.pop(T + 1)
            with nc.named_scope(f"B{T}"):
                for u in weave(bunits, aunits):
                    u()
        with nc.named_scope("Ctail"):
            for qb in (NT - 1,):
                for u in phase_c(qb):
                    u()


    split_multi_waits(nc)
    return nc


def kernel(hidden_states, attention_mask, position_ids, Wqkv, Wo):
    bf16 = ml_dtypes.bfloat16
    hidden = np.asarray(hidden_states, np.float32)
    S = hidden.shape[1]
    X = hidden[0]  # [S, HID]
    XT = np.ascontiguousarray(X.T).astype(bf16)  # [HID, S]
    # block-major device layout: [128, (T, kc, s)] (see _build)
    XTB = np.ascontiguousarray(
        XT.reshape(KC, 128, S // 512, 512).transpose(1, 2, 0, 3)
        .reshape(128, (S // 512) * KC * 512))

    pos = np.asarray(position_ids)[0].astype(np.float64)
    inv = 1.0 / (ROPE_THETA ** (np.arange(0, HD, 2, dtype=np.float64) / HD))
    freqs = inv[:, None] * pos[None, :]  # [128, S]
    cosT = np.cos(freqs).astype(np.float32)
    sinT = np.sin(freqs).astype(np.float32)

    plan, maskb = _classify_mask(attention_mask, S)

    Wqkv = np.asarray(Wqkv, np.float32)
    Wo = np.asarray(Wo, np.float32)

    one_cs = np.ones_like(cosT)
    zero_cs = np.zeros_like(sinT)

    in_maps = []
    for c in range(N_CORES):
        g = c // (NH // NKV)
        wq = Wqkv[c * HD:(c + 1) * HD] * SCALE  # exact: SCALE = 2**-4
        wk = Wqkv[NH * HD + g * HD: NH * HD + (g + 1) * HD]
        wv = Wqkv[(NH + NKV) * HD + g * HD: (NH + NKV) * HD + (g + 1) * HD]
        # even core of a pair projects+sends k (RoPE'd), odd projects+sends
        # v (coss=1/sins=0 makes the s-RoPE the identity)
        ws = wk if c % 2 == 0 else wv
        wqk = np.ascontiguousarray(
            np.concatenate([wq.T, ws.T], axis=1)).astype(bf16)
        wot = np.ascontiguousarray(Wo[:, c * HD:(c + 1) * HD].T).astype(bf16)
        in_maps.append({
            "xt": XTB, "wqk": wqk, "wo": wot,
            "cosq": cosT, "sinq": sinT,
            "coss": cosT if c % 2 == 0 else one_cs,
            "sins": sinT if c % 2 == 0 else zero_cs,
            "maskb": maskb,
        })

    nc = _build(S, plan, maskb.shape[0])
    res = run_bass_kernel_spmd(nc, in_maps, list(range(N_CORES)),
                               trace=TRACE)
    acc = res.results[0]["out"].astype(np.float32)
    for c in range(1, N_CORES):
        acc += res.results[c]["out"].astype(np.float32)
    # unshuffle the [t, fi, p, 512] block layout back to [S, HID]
    fbs = [(0, 512), (512, 512), (1024, 512), (1536, 512), (2048, 256)]
    out = np.empty((S, HID), np.float32)
    for fi, (f0, fw) in enumerate(fbs):
        out[:, f0:f0 + fw] = acc[:, fi, :, :fw].reshape(S, fw)
    kernel.last_exec_time_ns = res.exec_time_ns
    kernel.last_results = res
    return out[None].astype(np.float32)


kernel.last_exec_time_ns = None
kernel.last_results = None



# revision 44
# speedup vs baseline: 1.2781x; 1.0005x over previous
"""Gemma2 fused attention (B=1, S=4096, HID=2304, NH=8, NKV=4, HD=256,
sliding window 2048, softcap 50) on 8 Trainium2 NeuronCores.

Sharding: one query head per core (tensor parallel). Each GQA pair of
cores splits its shared k/v projection: the even core projects+RoPEs k,
the odd core projects v (identity "RoPE" via cos=1/sin=0 tables, so the
program stays SPMD-identical and only input data differs), then a
pairwise AllGather publishes [k | v] identically on both cores. The
received v ([hd, tok]) is PE-transposed into the [tok, hd] layout the
attnT matmuls need. o_proj is sharded over the contraction dim; bf16
per-core partials are summed on the host.

Per-core math (core c, head h=c, kv group g=c//2):
  qT,sT = (W @ X.T) in [head_dim, tok] layout, RoPE'd on device (bf16
  cos/sin tables from host; attention scale folded into Wq exactly).
  S.T[k,q] = kT.T @ qT; u = tanh(S.T/50); E = exp(50*u + mask) in bf16
  (softcap bounds logits to +-50 so no max-subtraction is needed).
  Mask handled per 128(k) x 512(q) block: all-zero blocks skip the add,
  fully-masked blocks are skipped entirely, mixed blocks add mask*0.02
  from a host-packed bf16 block stack, and fully-masked q-columns are
  trimmed from the matmuls/activations (data-driven, no pattern
  assumption).
  Z = ones.T @ E(bf16) as a PSUM row; attnT = (E @ v).T via lhsT=v.
  out_partial[tok, 2304] = attnT.T @ WoT with 1/Z fused into the
  PSUM->SBUF drain. Host unshuffles the block layout and sums partials.

Schedule: A (proj+exchange) runs two 512-token blocks ahead of B
(attention); exchange-consume units (recv + v-transpose, the first
PE-queue ops that wait on the collective) are woven a full B-phase
after their collective was issued; proj(T-1) units are woven into B_T
with the PSUM drain copy lagging one unit behind its matmuls; xt loads
are host-preshuffled block-major so each block is one contiguous DMA.
"""

import numpy as np
import ml_dtypes
from contextlib import ExitStack

import concourse.bass as bass
import concourse.tile as tile
import concourse.mybir as mybir
from concourse.bass_utils import run_bass_kernel_spmd
from concourse.masks import make_identity
from concourse.vector_clock import ScopedClock

N_CORES = 8
HID = 2304
NH, NKV, HD = 8, 4, 256
SCALE = 256.0 ** -0.5
SOFTCAP = 50.0
ROPE_THETA = 10000.0
KC = HID // 128  # 18 contraction chunks for the projections
CC_GROUPS = [[0, 1], [2, 3], [4, 5], [6, 7]]  # GQA pair exchange

BF16 = mybir.dt.bfloat16
F32 = mybir.dt.float32
AF = mybir.ActivationFunctionType

TRACE = False  # test harness flips this to get NTFF exec time


class TC(tile.TileContext):
    """TileContext whose final drain splits sem waits one-per-instruction
    (this walrus rejects instructions carrying more than one wait)."""

    def _drain_and_barrier(self, tick_clock, wait_clock):
        probe = self.nc.sync.nop(nofuse=True, hint="drain_waits")
        wait_clock.add_sem_waits(
            probe.ins, ScopedClock({None: tick_clock.global_clock})
        )
        waits = list(probe.ins.sync_info.on_wait)
        probe.ins.sync_info.on_wait = waits[:1]
        rest = waits[1:]
        while rest:
            extra = self.nc.sync.nop(nofuse=True, hint="drain_waits")
            extra.ins.sync_info = mybir.SyncInfo(on_wait=rest[:1], on_update=[])
            rest = rest[1:]
        self.nc.sync.drain()
        self.nc.all_engine_barrier()
        popped = self.nc._tile_sem_poison_stack.pop()
        assert popped is self._sem_poison
        self.nc.clear_and_free_semaphores(list(self.sems.allocated().values()))
        self.nc.all_engine_barrier()


def split_multi_waits(nc):
    """Split multi-wait instructions: extras move onto same-engine NoOps
    inserted immediately before (engines execute in program order)."""
    ctr = 0
    for f in nc.m.functions:
        for b in f.blocks:
            insts = list(b.instructions)
            new = []
            changed = False
            for inst in insts:
                si = inst.sync_info
                if si is not None and len(si.on_wait) > 1:
                    waits = list(si.on_wait)
                    for w in waits[:-1]:
                        ctr += 1
                        nop = mybir.InstNoOp(
                            name=f"I-waitsplit-{ctr}",
                            engine=inst.engine,
                            debug=inst.debug,
                            sync_info=mybir.SyncInfo(on_wait=[w], on_update=[]),
                        )
                        new.append(nop)
                    inst.sync_info = mybir.SyncInfo(
                        on_wait=[waits[-1]], on_update=list(si.on_update)
                    )
                    changed = True
                new.append(inst)
            if changed:
                b.instructions = new
    return ctr


def _classify_mask(mask, S):
    """Per (k-chunk 128, q-block 512) block: 'skip' (fully masked),
    'clean' (all zero) or mixed (apply additively). Each plan entry is
    (j, mix, q0, q1): only q-columns [q0, q1) have any unmasked k in the
    chunk, so S/E/attnT work is restricted to that slice. The first entry
    of every row is full-width so it can init the PSUM accumulation and
    zacc. Returns plan and the packed mixed-block stack (already scaled
    by 1/SOFTCAP)."""
    maskT = np.ascontiguousarray(np.asarray(mask, np.float32)[0, 0].T)  # [k, q]
    nj, nq = S // 128, S // 512
    blocks = maskT.reshape(nj, 128, nq, 512)
    mx = blocks.max(axis=(1, 3))
    mn = blocks.min(axis=(1, 3))
    skip = mx < -1e8
    clean = (mx == 0.0) & (mn == 0.0)
    plan = []
    mix_blocks = []
    for qb in range(nq):
        row = []
        for j in range(nj):
            if skip[j, qb]:
                continue
            if clean[j, qb]:
                row.append((j, -1, 0, 512))
            else:
                blk = maskT[j * 128:(j + 1) * 128, qb * 512:(qb + 1) * 512]
                col_ok = blk.max(axis=0) > -1e8
                q0 = int(col_ok.argmax())
                q1 = 512 - int(col_ok[::-1].argmax())
                if not col_ok[q0:q1].all():
                    q0, q1 = 0, 512  # non-contiguous valid span: no trim
                mix_blocks.append((blk * (1.0 / SOFTCAP)).astype(np.float32))
                row.append((j, len(mix_blocks) - 1, q0, q1))
        if not row:
            # fully-masked q-block (unreachable for causal masks): keep the
            # diagonal chunks so the PSUM accumulations are still defined
            for j in range(4 * qb, 4 * qb + 4):
                mix_blocks.append(
                    (maskT[j * 128:(j + 1) * 128, qb * 512:(qb + 1) * 512]
                     * (1.0 / SOFTCAP)).astype(np.float32))
                row.append((j, len(mix_blocks) - 1, 0, 512))
        # first entry must be full-width (inits PSUM + zacc). Prefer the
        # lowest-j full-width entry (oldest k/v data, longest ready) so the
        # high-j chunks — which depend on the freshest k/v exchange — run
        # last; else widen the first (safe for mixed entries: the additive
        # mask zeroes E outside the valid span).
        full = next((i for i, e in enumerate(row)
                     if e[2] == 0 and e[3] == 512), None)
        if full is None:
            j, mix, _, _ = row[0]
            assert mix >= 0
            row[0] = (j, mix, 0, 512)
        else:
            row = [row[full]] + row[:full] + row[full + 1:]
        plan.append(row)
    if mix_blocks:
        maskb = np.stack(mix_blocks)
    else:
        maskb = np.zeros((1, 128, 512), np.float32)
    return plan, maskb


def _build(S, plan, nmix):
    """Emit the SPMD program (identical for all cores; only data differs)."""
    NT = S // 512  # token/query 512-blocks
    nc = bass.Bass("TRN2", target_bir_lowering=False, debug=False,
                   num_devices=N_CORES)

    # xt is host-preshuffled block-major: xt_d[p, (T, kc, s)] =
    # X.T[kc*128 + p, T*512 + s], so each block's 2.25 MB loads as one
    # DMA of 128 contiguous 18 KB partition runs instead of 2304 1 KB
    # descriptors.
    NTb = S // 512
    xt_d = nc.dram_tensor("xt", [128, NTb * KC * 512], BF16,
                          kind="ExternalInput")
    wqk_d = nc.dram_tensor("wqk", [HID, 512], BF16, kind="ExternalInput")
    wo_d = nc.dram_tensor("wo", [256, HID], BF16, kind="ExternalInput")
    cosq_d = nc.dram_tensor("cosq", [128, S], F32, kind="ExternalInput")
    sinq_d = nc.dram_tensor("sinq", [128, S], F32, kind="ExternalInput")
    coss_d = nc.dram_tensor("coss", [128, S], F32, kind="ExternalInput")
    sins_d = nc.dram_tensor("sins", [128, S], F32, kind="ExternalInput")
    maskb_d = nc.dram_tensor("maskb", [nmix, 128, 512], F32,
                             kind="ExternalInput")
    # block-layout output (one fully-contiguous write per proj unit); the
    # host unshuffles back to [S, HID]. bf16 partials: each core's partial
    # is rounded once (~0.2% rms), the host sums in f32.
    out_d = nc.dram_tensor("out", [S // 128, 5, 128, 512], BF16,
                           kind="ExternalOutput")
    # pairwise k/v exchange: each core projects q plus ONE of (k, v) for
    # its GQA group ("s", in [hd, tok] layout); the pair AllGather makes
    # both halves visible as cc_out[T] = [k(2x128x512) | v(2x128x512)]
    # identically on both cores (rank order), so the consuming program is
    # parity-independent.
    cc_in = nc.dram_tensor("cc_in", [NT, 2, 128, 512], BF16, kind="Internal")
    cc_out = nc.dram_tensor("cc_out", [NT, 2, 2, 128, 512], BF16,
                            kind="Internal")

    with ExitStack() as ctx:
        tc = ctx.enter_context(TC(nc))
        P = lambda name, bufs, space="SBUF": ctx.enter_context(
            tc.tile_pool(name=name, bufs=bufs, space=space))

        wpool = P("w", 1)
        xpool = P("x", 3)
        cspool = P("cs", 2)
        qkpool = P("qk", 1)
        vpool = P("v", 1)
        spool = P("s", 2)
        vspool = P("vs", 2)
        tmppool = P("tmp", 4)
        upool = P("u", 4)
        epool = P("e", 10)
        mpool = P("m", 3)
        apool = P("a", 1)
        zpool = P("z", 2)
        zapool = P("za", 2)
        opool = P("o", 4)
        rpool = P("r", 1)
        dpool = P("d", 1, "DRAM")

        ps_qk = P("ps_qk", 3, "PSUM")
        ps_v = P("ps_v", 1, "PSUM")  # shared: proj tiles + transpose pads
        ps_s = P("ps_s", 2, "PSUM")
        ps_o = P("ps_o", 2, "PSUM")

        # --- resident weights / constants (wqk streams in chunk-
        # interleaved inside phase A block 0) ---
        wqk = wpool.tile([128, KC * 512], BF16, tag="wqk")
        wo = wpool.tile([128, 2 * HID], BF16, tag="wo")
        nc.sync.dma_start(
            wo[:].rearrange("p (c f) -> p c f", f=HID),
            wo_d[:, :].rearrange("(c p) f -> p c f", p=128))
        ones = wpool.tile([128, 1], BF16, tag="ones")
        nc.gpsimd.memset(ones[:], 1.0)
        ident = wpool.tile([128, 128], BF16, tag="ident")
        make_identity(nc, ident[:])

        xt1_tile = xpool.tile([128, KC * 512], BF16, tag="xt")

        # persistent activations (bf16, [128, S] each)
        qlo = qkpool.tile([128, S], BF16, tag="qlo")
        qhi = qkpool.tile([128, S], BF16, tag="qhi")
        klo = qkpool.tile([128, S], BF16, tag="klo")
        khi = qkpool.tile([128, S], BF16, tag="khi")
        vt = vpool.tile([128, (S // 128) * 256], BF16, tag="vt")
        alo = apool.tile([128, S], BF16, tag="alo")
        ahi = apool.tile([128, S], BF16, tag="ahi")
        rc = rpool.tile([128, S // 128], F32, tag="rc")
        zc = rpool.tile([128, S // 128], F32, tag="zc")
        zdram = dpool.tile([NT, 512], F32, tag="zdram")

        def phase_a(T):
            """Return emission units (closures) for QKV block T: project
            q + s (s = k on even cores, v on odd — distinguished purely by
            input data: the wqk second half and the coss/sins tables, which
            are identity for v), then exchange s within the core pair and
            transpose the received v into [tok, hd] layout."""
            c0 = T * 512
            if T == 1:
                xt = phase_a.xt1
            else:
                xt = xpool.tile([128, KC * 512], BF16, tag="xt")
                if T == 0:
                    phase_a.xt1 = xt1_tile
            slo = spool.tile([128, 512], BF16, tag="slo")
            shi = spool.tile([128, 512], BF16, tag="shi")
            qk_dst = [(qlo, c0), (qhi, c0), (slo, 0), (shi, 0)]
            units = []

            xb0 = T * KC * 512

            def dma_unit():
                if T == 0:
                    # chunk-interleaved so the first matmul only waits for
                    # chunk 0, not the whole 4.5 MB of weights+activations.
                    # Block 1's xt rides the same interleave: its late
                    # arrival otherwise delays A1 -> collective 1 -> B1's
                    # diagonal chunks.
                    for kc in range(KC):
                        nc.sync.dma_start(
                            wqk[:, kc * 512:(kc + 1) * 512],
                            wqk_d[kc * 128:(kc + 1) * 128, :])
                        nc.sync.dma_start(
                            xt[:, kc * 512:(kc + 1) * 512],
                            xt_d[:, xb0 + kc * 512: xb0 + (kc + 1) * 512])
                        nc.sync.dma_start(
                            xt1_tile[:, kc * 512:(kc + 1) * 512],
                            xt_d[:, KC * 512 + kc * 512:
                                 KC * 512 + (kc + 1) * 512])
                elif T == 1:
                    pass  # loaded by block 0's interleave
                else:
                    nc.sync.dma_start(
                        xt[:], xt_d[:, xb0:xb0 + KC * 512])
            units.append(dma_unit)

            cosq = cspool.tile([128, 512], F32, tag="cosq")
            sinq = cspool.tile([128, 512], F32, tag="sinq")
            coss = cspool.tile([128, 512], F32, tag="coss")
            sins = cspool.tile([128, 512], F32, tag="sins")
            cs_pair = [(cosq, sinq), (coss, sins)]

            def cs_unit():
                nc.sync.dma_start(cosq[:], cosq_d[:, c0:c0 + 512])
                nc.sync.dma_start(sinq[:], sinq_d[:, c0:c0 + 512])
                nc.sync.dma_start(coss[:], coss_d[:, c0:c0 + 512])
                nc.sync.dma_start(sins[:], sins_d[:, c0:c0 + 512])
            units.append(cs_unit)

            def rope_pair(plo, phi, pair):
                cos, sin = cs_pair[pair]
                (dlo, o), (dhi, _) = qk_dst[2 * pair], qk_dst[2 * pair + 1]
                t1 = tmppool.tile([128, 512], F32, tag="tmp")
                nc.vector.tensor_mul(t1[:], phi[:], sin[:])
                t2 = tmppool.tile([128, 512], F32, tag="tmp")
                nc.vector.tensor_mul(t2[:], plo[:], cos[:])
                nc.vector.tensor_sub(dlo[:, o:o + 512], t2[:], t1[:])
                t3 = tmppool.tile([128, 512], F32, tag="tmp")
                nc.vector.tensor_mul(t3[:], plo[:], sin[:])
                t4 = tmppool.tile([128, 512], F32, tag="tmp")
                nc.vector.tensor_mul(t4[:], phi[:], cos[:])
                nc.vector.tensor_add(dhi[:, o:o + 512], t4[:], t3[:])

            pp = {}

            def qk_unit(ft):
                ps = ps_qk.tile([128, 512], F32, tag="ps_qk")
                for kc in range(KC):
                    nc.tensor.matmul(
                        ps[:],
                        wqk[:, kc * 512 + ft * 128: kc * 512 + ft * 128 + 128],
                        xt[:, kc * 512:(kc + 1) * 512],
                        start=(kc == 0), stop=(kc == KC - 1))
                pp[ft] = ps
                if ft % 2 == 1:  # rotate the (lo, hi) pair
                    rope_pair(pp[ft - 1], pp[ft], ft // 2)

            def qk_chunk_major():
                # block 0 is paced by the weight/activation DMAs: keep 4
                # accumulations in flight (borrowing idle B-phase banks) so
                # each arriving chunk feeds 4 matmuls
                psA0 = ps_qk.tile([128, 512], F32, tag="ps_qk")
                psA1 = ps_qk.tile([128, 512], F32, tag="ps_qk")
                psA2 = ps_s.tile([128, 512], F32, tag="ps_s")
                psA3 = ps_o.tile([128, 512], F32, tag="ps_o")
                psA = [psA0, psA1, psA2, psA3]
                for kc in range(KC):
                    for ft in range(4):
                        nc.tensor.matmul(
                            psA[ft][:],
                            wqk[:, kc * 512 + ft * 128: kc * 512 + ft * 128 + 128],
                            xt[:, kc * 512:(kc + 1) * 512],
                            start=(kc == 0), stop=(kc == KC - 1))
                for pair in range(2):
                    rope_pair(psA[2 * pair], psA[2 * pair + 1], pair)

            if T == 0:
                units.append(qk_chunk_major)
            else:
                for ft in range(4):
                    units.append(lambda ft=ft: qk_unit(ft))

            def send_unit():
                nc.sync.dma_start(cc_in[T, 0, :, :], slo[:])
                nc.sync.dma_start(cc_in[T, 1, :, :], shi[:])

            def cc_unit():
                nc.gpsimd.collective_compute(
                    "AllGather", mybir.AluOpType.bypass, CC_GROUPS,
                    ins=[cc_in[T, :, :, :]], outs=[cc_out[T, :, :, :, :]])

            vsl = vspool.tile([128, 512], BF16, tag="vsl")
            vsh = vspool.tile([128, 512], BF16, tag="vsh")

            def recv_unit():
                nc.sync.dma_start(klo[:, c0:c0 + 512], cc_out[T, 0, 0, :, :])
                nc.sync.dma_start(khi[:, c0:c0 + 512], cc_out[T, 0, 1, :, :])
                nc.sync.dma_start(vsl[:], cc_out[T, 1, 0, :, :])
                nc.sync.dma_start(vsh[:], cc_out[T, 1, 1, :, :])

            def tr_unit(sub):
                tok = T * 4 + sub
                tp = ps_v.tile([128, 256], BF16, tag="ps_v")
                nc.tensor.transpose(tp[:, 0:128],
                                    vsl[:, sub * 128:(sub + 1) * 128],
                                    ident[:])
                nc.tensor.transpose(tp[:, 128:256],
                                    vsh[:, sub * 128:(sub + 1) * 128],
                                    ident[:])
                nc.vector.tensor_copy(vt[:, tok * 256:(tok + 1) * 256], tp[:])

            units.append(send_unit)
            units.append(cc_unit)
            consume = [recv_unit] + [lambda sub=sub: tr_unit(sub)
                                     for sub in range(4)]
            return units, consume

        def phase_b(qb):
            """Return emission units for attention q-block qb, one per
            k-chunk. The S matmuls of j lead the E-consumers of j-1 so the
            ACT chain has a full PE iteration of slack."""
            c0 = qb * 512
            zacc = zapool.tile([128, 512], F32, tag="za")
            olo = ps_o.tile([128, 512], F32, tag="ps_o")
            ohi = ps_o.tile([128, 512], F32, tag="ps_o")
            row = plan[qb]
            state = {}

            def s_unit(idx):
                j, mix, q0, q1 = row[idx]
                w = q1 - q0
                if mix >= 0:
                    mk = mpool.tile([128, 512], F32, tag="m")
                    nc.sync.dma_start(mk[:, :w], maskb_d[mix, :, q0:q1])
                else:
                    mk = None
                sps = ps_s.tile([128, 512], F32, tag="ps_s")
                nc.tensor.matmul(sps[:, q0:q1], klo[:, j * 128:(j + 1) * 128],
                                 qlo[:, c0 + q0:c0 + q1],
                                 start=True, stop=False)
                nc.tensor.matmul(sps[:, q0:q1], khi[:, j * 128:(j + 1) * 128],
                                 qhi[:, c0 + q0:c0 + q1],
                                 start=False, stop=True)
                e = epool.tile([128, 512], BF16, tag="e")
                u = upool.tile([128, 512], F32, tag="u")
                nc.scalar.activation(u[:, :w], sps[:, q0:q1], AF.Tanh,
                                     scale=1.0 / SOFTCAP)
                if mk is not None:
                    u2 = upool.tile([128, 512], F32, tag="u")
                    nc.vector.tensor_add(u2[:, :w], u[:, :w], mk[:, :w])
                    u = u2
                nc.scalar.activation(e[:, q0:q1], u[:, :w], AF.Exp,
                                     scale=SOFTCAP)
                if idx == 0:
                    nc.vector.tensor_copy(zacc[:], e[:])
                else:
                    nc.vector.tensor_add(zacc[:, q0:q1], zacc[:, q0:q1],
                                         e[:, q0:q1])
                state[idx] = e

            def mm_unit(idx):
                j, _, q0, q1 = row[idx]
                e = state.pop(idx)
                first, last = idx == 0, idx == len(row) - 1
                nc.tensor.matmul(olo[:, q0:q1], vt[:, j * 256:j * 256 + 128],
                                 e[:, q0:q1], start=first, stop=last)
                nc.tensor.matmul(ohi[:, q0:q1],
                                 vt[:, j * 256 + 128:(j + 1) * 256],
                                 e[:, q0:q1], start=first, stop=last)

            def tail_unit():
                nc.vector.tensor_copy(alo[:, c0:c0 + 512], olo[:])
                nc.vector.tensor_copy(ahi[:, c0:c0 + 512], ohi[:])
                # single f32->bf16 rounding of zacc so the 128-partition
                # reduction runs as a 1-cycle/row bf16 matmul (fp32 is 4x)
                zaccb = zpool.tile([128, 512], BF16, tag="zb")
                nc.scalar.copy(zaccb[:], zacc[:])
                zps = ps_s.tile([1, 512], F32, tag="ps_s")
                nc.tensor.matmul(zps[:], ones[:], zaccb[:],
                                 start=True, stop=True)
                zrow = zpool.tile([1, 512], F32, tag="z")
                nc.vector.tensor_copy(zrow[:], zps[:])
                nc.sync.dma_start(zdram[qb, :], zrow[:])
                # incremental 1/Z so proj for this q-block can start now
                nc.sync.dma_start(
                    zc[:, 4 * qb:4 * qb + 4],
                    zdram[qb, :].rearrange("(b p) -> p b", p=128))
                nc.vector.reciprocal(rc[:, 4 * qb:4 * qb + 4],
                                     zc[:, 4 * qb:4 * qb + 4])

            units = [lambda: s_unit(0)]
            for idx in range(1, len(row)):
                units.append(lambda idx=idx: (s_unit(idx), mm_unit(idx - 1)))
            units.append(lambda: (mm_unit(len(row) - 1), tail_unit()))
            return units

        # PE warmup: a few throwaway matmuls so HAM reaches 8/8 before
        # the first real accumulation
        scratch = wpool.tile([128, 512], BF16, tag="scratch")
        nc.vector.memset(scratch[:], 0.0)
        wps = ps_s.tile([128, 512], F32, tag="ps_s")
        for _ in range(12):
            nc.tensor.matmul(wps[:], scratch[:, :128], scratch[:],
                             start=True, stop=True)

        # output projection units (one per (tok-tile, feat-block)); the
        # 1/Z normalization is fused into the PSUM->SBUF copy. These are
        # woven into later B phases so the 37 MB output DMA spreads over
        # the whole kernel instead of saturating the tail.
        fbs = [(0, 512), (512, 512), (1024, 512), (1536, 512), (2048, 256)]

        def proj_mm(t, fi, st):
            f0, fw = fbs[fi]
            pool = ps_qk if fi % 3 < 2 else ps_v
            ps = pool.tile([128, 512], F32, tag=pool.name)
            nc.tensor.matmul(ps[:, :fw], alo[:, t * 128:(t + 1) * 128],
                             wo[:, f0:f0 + fw], start=True, stop=False)
            nc.tensor.matmul(ps[:, :fw], ahi[:, t * 128:(t + 1) * 128],
                             wo[:, HID + f0:HID + f0 + fw],
                             start=False, stop=True)
            st[(t, fi)] = ps

        def proj_cp(t, fi, st):
            f0, fw = fbs[fi]
            ps = st.pop((t, fi))
            osb = opool.tile([128, 512], BF16, tag="o")
            if fi % 2 == 0 and fi < 4:
                nc.scalar.activation(osb[:, :fw], ps[:, :fw], AF.Copy,
                                     scale=rc[:, t:t + 1])
            else:
                nc.vector.tensor_scalar_mul(osb[:, :fw], ps[:, :fw],
                                            rc[:, t:t + 1])
            nc.sync.dma_start(out_d[t, fi, :, :fw], osb[:, :fw])

        def phase_c(qb):
            # drain copy of proj i-1 rides with the matmuls of proj i, so
            # each copy has a full weave-slot of engine-queue slack before
            # its PSUM bank is recycled
            prs = [(t, fi) for t in range(4 * qb, 4 * qb + 4)
                   for fi in range(len(fbs))]
            st = {}

            def unit(i):
                if i > 0:
                    proj_cp(*prs[i - 1], st)
                proj_mm(*prs[i], st)

            units = [lambda i=i: unit(i) for i in range(len(prs))]
            units.append(lambda: proj_cp(*prs[-1], st))
            return units

        def weave(bunits, aunits):
            """Alternate B and A units so stalled B consumers never block
            independent A matmuls in the in-order PE queue."""
            out = []
            na, nb = len(aunits), len(bunits)
            ai = 0
            for bi, bu in enumerate(bunits):
                out.append(bu)
                want = (bi + 1) * na // nb
                while ai < want:
                    out.append(aunits[ai])
                    ai += 1
            out.extend(aunits[ai:])
            return out

        # A runs two blocks ahead of B; each block's exchange-consume units
        # (recv DMAs + v transposes, the first PE-queue instructions that
        # WAIT on the collective) are woven a full B-phase after the
        # collective was issued, so the pair AllGather never stalls the
        # in-order PE queue.
        consumes = {}
        prod0, consumes[0] = phase_a(0)
        prod1, consumes[1] = phase_a(1)
        # prologue order: both xt DMAs first, A0 compute, then A0's
        # send+collective immediately (so cc_0 runs during A1 compute),
        # then A1, then consume(0)
        for u in ([prod0[0], prod0[1], prod1[0]] + prod0[2:]
                  + [prod1[1]] + prod1[2:] + consumes.pop(0)):
            u()
        for T in range(NT):
            bunits = phase_b(T)
            aunits = []
            if T + 2 < NT:
                prod, consumes[T + 2] = phase_a(T + 2)
                aunits += prod
            if T >= 1:
                aunits += phase_c(T - 1)
            if T + 1 in consumes:
                aunits += consumes.pop(T + 1)
            with nc.named_scope(f"B{T}"):
                for u in weave(bunits, aunits):
                    u()
        with nc.named_scope("Ctail"):
            for qb in (NT - 1,):
                for u in phase_c(qb):
                    u()


    split_multi_waits(nc)
    return nc


def kernel(hidden_states, attention_mask, position_ids, Wqkv, Wo):
    bf16 = ml_dtypes.bfloat16
    hidden = np.asarray(hidden_states, np.float32)
    S = hidden.shape[1]
    X = hidden[0]  # [S, HID]
    XT = np.ascontiguousarray(X.T).astype(bf16)  # [HID, S]
    # block-major device layout: [128, (T, kc, s)] (see _build)
    XTB = np.ascontiguousarray(
        XT.reshape(KC, 128, S // 512, 512).transpose(1, 2, 0, 3)
        .reshape(128, (S // 512) * KC * 512))

    pos = np.asarray(position_ids)[0].astype(np.float64)
    inv = 1.0 / (ROPE_THETA ** (np.arange(0, HD, 2, dtype=np.float64) / HD))
    freqs = inv[:, None] * pos[None, :]  # [128, S]
    cosT = np.cos(freqs).astype(np.float32)
    sinT = np.sin(freqs).astype(np.float32)

    plan, maskb = _classify_mask(attention_mask, S)

    Wqkv = np.asarray(Wqkv, np.float32)
    Wo = np.asarray(Wo, np.float32)

    one_cs = np.ones_like(cosT)
    zero_cs = np.zeros_like(sinT)

    in_maps = []
    for c in range(N_CORES):
        g = c // (NH // NKV)
        wq = Wqkv[c * HD:(c + 1) * HD] * SCALE  # exact: SCALE = 2**-4
        wk = Wqkv[NH * HD + g * HD: NH * HD + (g + 1) * HD]
        wv = Wqkv[(NH + NKV) * HD + g * HD: (NH + NKV) * HD + (g + 1) * HD]
        # even core of a pair projects+sends k (RoPE'd), odd projects+sends
        # v (coss=1/sins=0 makes the s-RoPE the identity)
        ws = wk if c % 2 == 0 else wv
        wqk = np.ascontiguousarray(
            np.concatenate([wq.T, ws.T], axis=1)).astype(bf16)
        wot = np.ascontiguousarray(Wo[:, c * HD:(c + 1) * HD].T).astype(bf16)
        in_maps.append({
            "xt": XTB, "wqk": wqk, "wo": wot,
            "cosq": cosT, "sinq": sinT,
            "coss": cosT if c % 2 == 0 else one_cs,
            "sins": sinT if c % 2 == 0 else zero_cs,
            "maskb": maskb,
        })

    nc = _build(S, plan, maskb.shape[0])
    res = run_bass_kernel_spmd(nc, in_maps, list(range(N_CORES)),
                               trace=TRACE)
    acc = res.results[0]["out"].astype(np.float32)
    for c in range(1, N_CORES):
        acc += res.results[c]["out"].astype(np.float32)
    # unshuffle the [t, fi, p, 512] block layout back to [S, HID]
    fbs = [(0, 512), (512, 512), (1024, 512), (1536, 512), (2048, 256)]
    out = np.empty((S, HID), np.float32)
    for fi, (f0, fw) in enumerate(fbs):
        out[:, f0:f0 + fw] = acc[:, fi, :, :fw].reshape(S, fw)
    kernel.last_exec_time_ns = res.exec_time_ns
    kernel.last_results = res
    return out[None].astype(np.float32)


kernel.last_exec_time_ns = None
kernel.last_results = None



# revision 45
# speedup vs baseline: 1.3373x; 1.0464x over previous
"""Gemma2 fused attention (B=1, S=4096, HID=2304, NH=8, NKV=4, HD=256,
sliding window 2048, softcap 50) on 8 Trainium2 NeuronCores.

Sharding: one query head per core (tensor parallel). Each GQA pair of
cores splits its shared k/v projection: the even core projects+RoPEs k,
the odd core projects v (identity "RoPE" via cos=1/sin=0 tables, so the
program stays SPMD-identical and only input data differs), then a
pairwise AllGather publishes [k | v] identically on both cores. The
received v ([hd, tok]) is PE-transposed into the [tok, hd] layout the
attnT matmuls need. o_proj is sharded over the contraction dim; bf16
per-core partials are summed on the host.

Per-core math (core c, head h=c, kv group g=c//2):
  qT,sT = (W @ X.T) in [head_dim, tok] layout, RoPE'd on device (bf16
  cos/sin tables from host; attention scale folded into Wq exactly).
  S.T[k,q] = kT.T @ qT; u = tanh(S.T/50); E = exp(50*u + mask) in bf16
  (softcap bounds logits to +-50 so no max-subtraction is needed).
  Mask handled per 128(k) x 512(q) block: all-zero blocks skip the add,
  fully-masked blocks are skipped entirely, mixed blocks add mask*0.02
  from a host-packed bf16 block stack, and fully-masked q-columns are
  trimmed from the matmuls/activations (data-driven, no pattern
  assumption).
  Z = ones.T @ E(bf16) as a PSUM row; attnT = (E @ v).T via lhsT=v.
  out_partial[tok, 2304] = attnT.T @ WoT with 1/Z fused into the
  PSUM->SBUF drain. Host unshuffles the block layout and sums partials.

Schedule: A (proj+exchange) runs two 512-token blocks ahead of B
(attention); exchange-consume units (recv + v-transpose, the first
PE-queue ops that wait on the collective) are woven a full B-phase
after their collective was issued; proj(T-1) units are woven into B_T
with the PSUM drain copy lagging one unit behind its matmuls; xt loads
are host-preshuffled block-major so each block is one contiguous DMA.
"""

import numpy as np
import ml_dtypes
from contextlib import ExitStack

import concourse.bass as bass
import concourse.tile as tile
import concourse.mybir as mybir
from concourse.bass_utils import run_bass_kernel_spmd
from concourse.masks import make_identity
from concourse.vector_clock import ScopedClock

N_CORES = 8
HID = 2304
NH, NKV, HD = 8, 4, 256
SCALE = 256.0 ** -0.5
SOFTCAP = 50.0
ROPE_THETA = 10000.0
KC = HID // 128  # 18 contraction chunks for the projections
CC_GROUPS = [[0, 1], [2, 3], [4, 5], [6, 7]]  # GQA pair exchange

BF16 = mybir.dt.bfloat16
F32 = mybir.dt.float32
AF = mybir.ActivationFunctionType

TRACE = False  # test harness flips this to get NTFF exec time


class TC(tile.TileContext):
    """TileContext whose final drain splits sem waits one-per-instruction
    (this walrus rejects instructions carrying more than one wait)."""

    def _drain_and_barrier(self, tick_clock, wait_clock):
        probe = self.nc.sync.nop(nofuse=True, hint="drain_waits")
        wait_clock.add_sem_waits(
            probe.ins, ScopedClock({None: tick_clock.global_clock})
        )
        waits = list(probe.ins.sync_info.on_wait)
        probe.ins.sync_info.on_wait = waits[:1]
        rest = waits[1:]
        while rest:
            extra = self.nc.sync.nop(nofuse=True, hint="drain_waits")
            extra.ins.sync_info = mybir.SyncInfo(on_wait=rest[:1], on_update=[])
            rest = rest[1:]
        self.nc.sync.drain()
        self.nc.all_engine_barrier()
        popped = self.nc._tile_sem_poison_stack.pop()
        assert popped is self._sem_poison
        self.nc.clear_and_free_semaphores(list(self.sems.allocated().values()))
        self.nc.all_engine_barrier()


def split_multi_waits(nc):
    """Split multi-wait instructions: extras move onto same-engine NoOps
    inserted immediately before (engines execute in program order)."""
    ctr = 0
    for f in nc.m.functions:
        for b in f.blocks:
            insts = list(b.instructions)
            new = []
            changed = False
            for inst in insts:
                si = inst.sync_info
                if si is not None and len(si.on_wait) > 1:
                    waits = list(si.on_wait)
                    for w in waits[:-1]:
                        ctr += 1
                        nop = mybir.InstNoOp(
                            name=f"I-waitsplit-{ctr}",
                            engine=inst.engine,
                            debug=inst.debug,
                            sync_info=mybir.SyncInfo(on_wait=[w], on_update=[]),
                        )
                        new.append(nop)
                    inst.sync_info = mybir.SyncInfo(
                        on_wait=[waits[-1]], on_update=list(si.on_update)
                    )
                    changed = True
                new.append(inst)
            if changed:
                b.instructions = new
    return ctr


def _classify_mask(mask, S):
    """Per (k-chunk 128, q-block 512) block: 'skip' (fully masked),
    'clean' (all zero) or mixed (apply additively). Each plan entry is
    (j, mix, q0, q1): only q-columns [q0, q1) have any unmasked k in the
    chunk, so S/E/attnT work is restricted to that slice. The first entry
    of every row is full-width so it can init the PSUM accumulation and
    zacc. Returns plan and the packed mixed-block stack (already scaled
    by 1/SOFTCAP)."""
    maskT = np.ascontiguousarray(np.asarray(mask, np.float32)[0, 0].T)  # [k, q]
    nj, nq = S // 128, S // 512
    blocks = maskT.reshape(nj, 128, nq, 512)
    mx = blocks.max(axis=(1, 3))
    mn = blocks.min(axis=(1, 3))
    skip = mx < -1e8
    clean = (mx == 0.0) & (mn == 0.0)
    plan = []
    mix_blocks = []
    for qb in range(nq):
        row = []
        for j in range(nj):
            if skip[j, qb]:
                continue
            if clean[j, qb]:
                row.append((j, -1, 0, 512))
            else:
                blk = maskT[j * 128:(j + 1) * 128, qb * 512:(qb + 1) * 512]
                col_ok = blk.max(axis=0) > -1e8
                q0 = int(col_ok.argmax())
                q1 = 512 - int(col_ok[::-1].argmax())
                if not col_ok[q0:q1].all():
                    q0, q1 = 0, 512  # non-contiguous valid span: no trim
                mix_blocks.append((blk * (1.0 / SOFTCAP)).astype(np.float32))
                row.append((j, len(mix_blocks) - 1, q0, q1))
        if not row:
            # fully-masked q-block (unreachable for causal masks): keep the
            # diagonal chunks so the PSUM accumulations are still defined
            for j in range(4 * qb, 4 * qb + 4):
                mix_blocks.append(
                    (maskT[j * 128:(j + 1) * 128, qb * 512:(qb + 1) * 512]
                     * (1.0 / SOFTCAP)).astype(np.float32))
                row.append((j, len(mix_blocks) - 1, 0, 512))
        # first entry must be full-width (inits PSUM + zacc). Prefer the
        # lowest-j full-width entry (oldest k/v data, longest ready) so the
        # high-j chunks — which depend on the freshest k/v exchange — run
        # last; else widen the first (safe for mixed entries: the additive
        # mask zeroes E outside the valid span).
        full = next((i for i, e in enumerate(row)
                     if e[2] == 0 and e[3] == 512), None)
        if full is None:
            j, mix, _, _ = row[0]
            assert mix >= 0
            row[0] = (j, mix, 0, 512)
        else:
            row = [row[full]] + row[:full] + row[full + 1:]
        plan.append(row)
    if mix_blocks:
        maskb = np.stack(mix_blocks)
    else:
        maskb = np.zeros((1, 128, 512), np.float32)
    return plan, maskb


def _build(S, plan, nmix):
    """Emit the SPMD program (identical for all cores; only data differs)."""
    NT = S // 512  # token/query 512-blocks
    nc = bass.Bass("TRN2", target_bir_lowering=False, debug=False,
                   num_devices=N_CORES)

    # xt is host-preshuffled block-major: xt_d[p, (T, kc, s)] =
    # X.T[kc*128 + p, T*512 + s], so each block's 2.25 MB loads as one
    # DMA of 128 contiguous 18 KB partition runs instead of 2304 1 KB
    # descriptors.
    NTb = S // 512
    xt_d = nc.dram_tensor("xt", [128, NTb * KC * 512], BF16,
                          kind="ExternalInput")
    wqk_d = nc.dram_tensor("wqk", [HID, 512], BF16, kind="ExternalInput")
    wo_d = nc.dram_tensor("wo", [256, HID], BF16, kind="ExternalInput")
    cosq_d = nc.dram_tensor("cosq", [128, S], F32, kind="ExternalInput")
    sinq_d = nc.dram_tensor("sinq", [128, S], F32, kind="ExternalInput")
    coss_d = nc.dram_tensor("coss", [128, S], F32, kind="ExternalInput")
    sins_d = nc.dram_tensor("sins", [128, S], F32, kind="ExternalInput")
    maskb_d = nc.dram_tensor("maskb", [nmix, 128, 512], F32,
                             kind="ExternalInput")
    # block-layout output (one fully-contiguous write per proj unit); the
    # host unshuffles back to [S, HID]. bf16 partials: each core's partial
    # is rounded once (~0.2% rms), the host sums in f32.
    out_d = nc.dram_tensor("out", [S // 128, 5, 128, 512], BF16,
                           kind="ExternalOutput")
    # pairwise k/v exchange: each core projects q plus ONE of (k, v) for
    # its GQA group ("s", in [hd, tok] layout); the pair AllGather makes
    # both halves visible as cc_out[T] = [k(2x128x512) | v(2x128x512)]
    # identically on both cores (rank order), so the consuming program is
    # parity-independent.
    cc_in = nc.dram_tensor("cc_in", [NT, 2, 128, 512], BF16, kind="Internal")
    cc_out = nc.dram_tensor("cc_out", [NT, 2, 2, 128, 512], BF16,
                            kind="Internal")

    with ExitStack() as ctx:
        tc = ctx.enter_context(TC(nc))
        P = lambda name, bufs, space="SBUF": ctx.enter_context(
            tc.tile_pool(name=name, bufs=bufs, space=space))

        wpool = P("w", 1)
        xpool = P("x", 3)
        cspool = P("cs", 2)
        qkpool = P("qk", 1)
        vpool = P("v", 1)
        spool = P("s", 2)
        vspool = P("vs", 2)
        tmppool = P("tmp", 4)
        upool = P("u", 4)
        epool = P("e", 10)
        mpool = P("m", 3)
        apool = P("a", 1)
        zpool = P("z", 2)
        zapool = P("za", 2)
        opool = P("o", 4)
        rpool = P("r", 1)
        dpool = P("d", 1, "DRAM")

        ps_qk = P("ps_qk", 3, "PSUM")
        ps_v = P("ps_v", 1, "PSUM")  # shared: proj tiles + transpose pads
        ps_s = P("ps_s", 2, "PSUM")
        ps_o = P("ps_o", 2, "PSUM")

        # --- resident weights / constants (wqk streams in chunk-
        # interleaved inside phase A block 0) ---
        wqk = wpool.tile([128, KC * 512], BF16, tag="wqk")
        wo = wpool.tile([128, 2 * HID], BF16, tag="wo")
        nc.sync.dma_start(
            wo[:].rearrange("p (c f) -> p c f", f=HID),
            wo_d[:, :].rearrange("(c p) f -> p c f", p=128))
        ones = wpool.tile([128, 1], BF16, tag="ones")
        nc.gpsimd.memset(ones[:], 1.0)
        ident = wpool.tile([128, 128], BF16, tag="ident")
        make_identity(nc, ident[:])

        xt1_tile = xpool.tile([128, KC * 512], BF16, tag="xt")

        # persistent activations (bf16, [128, S] each)
        qlo = qkpool.tile([128, S], BF16, tag="qlo")
        qhi = qkpool.tile([128, S], BF16, tag="qhi")
        klo = qkpool.tile([128, S], BF16, tag="klo")
        khi = qkpool.tile([128, S], BF16, tag="khi")
        vt = vpool.tile([128, (S // 128) * 256], BF16, tag="vt")
        alo = apool.tile([128, S], BF16, tag="alo")
        ahi = apool.tile([128, S], BF16, tag="ahi")
        rc = rpool.tile([128, S // 128], F32, tag="rc")
        zc = rpool.tile([128, S // 128], F32, tag="zc")
        zdram = dpool.tile([NT, 512], F32, tag="zdram")

        def phase_a(T):
            """Return emission units (closures) for QKV block T: project
            q + s (s = k on even cores, v on odd — distinguished purely by
            input data: the wqk second half and the coss/sins tables, which
            are identity for v), then exchange s within the core pair and
            transpose the received v into [tok, hd] layout."""
            c0 = T * 512
            if T == 1:
                xt = phase_a.xt1
            else:
                xt = xpool.tile([128, KC * 512], BF16, tag="xt")
                if T == 0:
                    phase_a.xt1 = xt1_tile
            slo = spool.tile([128, 512], BF16, tag="slo")
            shi = spool.tile([128, 512], BF16, tag="shi")
            qk_dst = [(qlo, c0), (qhi, c0), (slo, 0), (shi, 0)]
            units = []

            xb0 = T * KC * 512

            def dma_unit():
                if T == 0:
                    # chunk-interleaved so the first matmul only waits for
                    # chunk 0, not the whole 4.5 MB of weights+activations.
                    # Block 1's xt rides the same interleave: its late
                    # arrival otherwise delays A1 -> collective 1 -> B1's
                    # diagonal chunks.
                    for kc in range(KC):
                        nc.sync.dma_start(
                            wqk[:, kc * 512:(kc + 1) * 512],
                            wqk_d[kc * 128:(kc + 1) * 128, :])
                        nc.sync.dma_start(
                            xt[:, kc * 512:(kc + 1) * 512],
                            xt_d[:, xb0 + kc * 512: xb0 + (kc + 1) * 512])
                        nc.sync.dma_start(
                            xt1_tile[:, kc * 512:(kc + 1) * 512],
                            xt_d[:, KC * 512 + kc * 512:
                                 KC * 512 + (kc + 1) * 512])
                elif T == 1:
                    pass  # loaded by block 0's interleave
                else:
                    nc.sync.dma_start(
                        xt[:], xt_d[:, xb0:xb0 + KC * 512])
            units.append(dma_unit)

            cosq = cspool.tile([128, 512], F32, tag="cosq")
            sinq = cspool.tile([128, 512], F32, tag="sinq")
            coss = cspool.tile([128, 512], F32, tag="coss")
            sins = cspool.tile([128, 512], F32, tag="sins")
            cs_pair = [(cosq, sinq), (coss, sins)]

            def cs_unit():
                nc.sync.dma_start(cosq[:], cosq_d[:, c0:c0 + 512])
                nc.sync.dma_start(sinq[:], sinq_d[:, c0:c0 + 512])
                nc.sync.dma_start(coss[:], coss_d[:, c0:c0 + 512])
                nc.sync.dma_start(sins[:], sins_d[:, c0:c0 + 512])
            units.append(cs_unit)

            def rope_pair(plo, phi, pair):
                cos, sin = cs_pair[pair]
                (dlo, o), (dhi, _) = qk_dst[2 * pair], qk_dst[2 * pair + 1]
                t1 = tmppool.tile([128, 512], F32, tag="tmp")
                nc.vector.tensor_mul(t1[:], phi[:], sin[:])
                t2 = tmppool.tile([128, 512], F32, tag="tmp")
                nc.vector.tensor_mul(t2[:], plo[:], cos[:])
                nc.vector.tensor_sub(dlo[:, o:o + 512], t2[:], t1[:])
                t3 = tmppool.tile([128, 512], F32, tag="tmp")
                nc.vector.tensor_mul(t3[:], plo[:], sin[:])
                t4 = tmppool.tile([128, 512], F32, tag="tmp")
                nc.vector.tensor_mul(t4[:], phi[:], cos[:])
                nc.vector.tensor_add(dhi[:, o:o + 512], t4[:], t3[:])

            pp = {}

            def qk_unit(ft):
                ps = ps_qk.tile([128, 512], F32, tag="ps_qk")
                for kc in range(KC):
                    nc.tensor.matmul(
                        ps[:],
                        wqk[:, kc * 512 + ft * 128: kc * 512 + ft * 128 + 128],
                        xt[:, kc * 512:(kc + 1) * 512],
                        start=(kc == 0), stop=(kc == KC - 1))
                pp[ft] = ps
                if ft % 2 == 1:  # rotate the (lo, hi) pair
                    rope_pair(pp[ft - 1], pp[ft], ft // 2)

            def qk_chunk_major():
                # block 0 is paced by the weight/activation DMAs: keep 4
                # accumulations in flight (borrowing idle B-phase banks) so
                # each arriving chunk feeds 4 matmuls
                psA0 = ps_qk.tile([128, 512], F32, tag="ps_qk")
                psA1 = ps_qk.tile([128, 512], F32, tag="ps_qk")
                psA2 = ps_s.tile([128, 512], F32, tag="ps_s")
                psA3 = ps_o.tile([128, 512], F32, tag="ps_o")
                psA = [psA0, psA1, psA2, psA3]
                for kc in range(KC):
                    for ft in range(4):
                        nc.tensor.matmul(
                            psA[ft][:],
                            wqk[:, kc * 512 + ft * 128: kc * 512 + ft * 128 + 128],
                            xt[:, kc * 512:(kc + 1) * 512],
                            start=(kc == 0), stop=(kc == KC - 1))
                for pair in range(2):
                    rope_pair(psA[2 * pair], psA[2 * pair + 1], pair)

            if T == 0:
                units.append(qk_chunk_major)
            else:
                for ft in range(4):
                    units.append(lambda ft=ft: qk_unit(ft))

            def send_unit():
                nc.sync.dma_start(cc_in[T, 0, :, :], slo[:])
                nc.sync.dma_start(cc_in[T, 1, :, :], shi[:])

            def cc_unit():
                nc.gpsimd.collective_compute(
                    "AllGather", mybir.AluOpType.bypass, CC_GROUPS,
                    ins=[cc_in[T, :, :, :]], outs=[cc_out[T, :, :, :, :]])

            vsl = vspool.tile([128, 512], BF16, tag="vsl")
            vsh = vspool.tile([128, 512], BF16, tag="vsh")

            def recv_unit():
                nc.sync.dma_start(klo[:, c0:c0 + 512], cc_out[T, 0, 0, :, :])
                nc.sync.dma_start(khi[:, c0:c0 + 512], cc_out[T, 0, 1, :, :])
                nc.sync.dma_start(vsl[:], cc_out[T, 1, 0, :, :])
                nc.sync.dma_start(vsh[:], cc_out[T, 1, 1, :, :])

            def tr_unit(sub):
                tok = T * 4 + sub
                tp = ps_v.tile([128, 256], BF16, tag="ps_v")
                nc.tensor.transpose(tp[:, 0:128],
                                    vsl[:, sub * 128:(sub + 1) * 128],
                                    ident[:])
                nc.tensor.transpose(tp[:, 128:256],
                                    vsh[:, sub * 128:(sub + 1) * 128],
                                    ident[:])
                nc.vector.tensor_copy(vt[:, tok * 256:(tok + 1) * 256], tp[:])

            units.append(send_unit)
            units.append(cc_unit)
            consume = [recv_unit] + [lambda sub=sub: tr_unit(sub)
                                     for sub in range(4)]
            return units, consume

        def phase_b(qb):
            """Return emission units for attention q-block qb, one per
            k-chunk. The S matmuls of j lead the E-consumers of j-1 so the
            ACT chain has a full PE iteration of slack."""
            c0 = qb * 512
            zacc = zapool.tile([128, 512], F32, tag="za")
            olo = ps_o.tile([128, 512], F32, tag="ps_o")
            ohi = ps_o.tile([128, 512], F32, tag="ps_o")
            row = plan[qb]
            state = {}

            def s_unit(idx):
                j, mix, q0, q1 = row[idx]
                w = q1 - q0
                if mix >= 0:
                    mk = mpool.tile([128, 512], F32, tag="m")
                    nc.sync.dma_start(mk[:, :w], maskb_d[mix, :, q0:q1])
                else:
                    mk = None
                sps = ps_s.tile([128, 512], F32, tag="ps_s")
                nc.tensor.matmul(sps[:, q0:q1], klo[:, j * 128:(j + 1) * 128],
                                 qlo[:, c0 + q0:c0 + q1],
                                 start=True, stop=False)
                nc.tensor.matmul(sps[:, q0:q1], khi[:, j * 128:(j + 1) * 128],
                                 qhi[:, c0 + q0:c0 + q1],
                                 start=False, stop=True)
                e = epool.tile([128, 512], BF16, tag="e")
                u = upool.tile([128, 512], F32, tag="u")
                nc.scalar.activation(u[:, :w], sps[:, q0:q1], AF.Tanh,
                                     scale=1.0 / SOFTCAP)
                if mk is not None:
                    u2 = upool.tile([128, 512], F32, tag="u")
                    nc.vector.tensor_add(u2[:, :w], u[:, :w], mk[:, :w])
                    u = u2
                nc.scalar.activation(e[:, q0:q1], u[:, :w], AF.Exp,
                                     scale=SOFTCAP)
                if idx == 0:
                    nc.vector.tensor_copy(zacc[:], e[:])
                else:
                    nc.vector.tensor_add(zacc[:, q0:q1], zacc[:, q0:q1],
                                         e[:, q0:q1])
                state[idx] = e

            def mm_unit(idx):
                j, _, q0, q1 = row[idx]
                e = state.pop(idx)
                first, last = idx == 0, idx == len(row) - 1
                nc.tensor.matmul(olo[:, q0:q1], vt[:, j * 256:j * 256 + 128],
                                 e[:, q0:q1], start=first, stop=last)
                nc.tensor.matmul(ohi[:, q0:q1],
                                 vt[:, j * 256 + 128:(j + 1) * 256],
                                 e[:, q0:q1], start=first, stop=last)

            def tail_unit():
                nc.vector.tensor_copy(alo[:, c0:c0 + 512], olo[:])
                nc.vector.tensor_copy(ahi[:, c0:c0 + 512], ohi[:])
                # single f32->bf16 rounding of zacc so the 128-partition
                # reduction runs as a 1-cycle/row bf16 matmul (fp32 is 4x)
                zaccb = zpool.tile([128, 512], BF16, tag="zb")
                nc.scalar.copy(zaccb[:], zacc[:])
                zps = ps_s.tile([1, 512], F32, tag="ps_s")
                nc.tensor.matmul(zps[:], ones[:], zaccb[:],
                                 start=True, stop=True)
                zrow = zpool.tile([1, 512], F32, tag="z")
                nc.vector.tensor_copy(zrow[:], zps[:])
                nc.sync.dma_start(zdram[qb, :], zrow[:])
                # incremental 1/Z so proj for this q-block can start now
                nc.sync.dma_start(
                    zc[:, 4 * qb:4 * qb + 4],
                    zdram[qb, :].rearrange("(b p) -> p b", p=128))
                nc.vector.reciprocal(rc[:, 4 * qb:4 * qb + 4],
                                     zc[:, 4 * qb:4 * qb + 4])

            units = [lambda: s_unit(0)]
            for idx in range(1, len(row)):
                units.append(lambda idx=idx: (s_unit(idx), mm_unit(idx - 1)))
            units.append(lambda: (mm_unit(len(row) - 1), tail_unit()))
            return units

        # PE warmup: a few throwaway matmuls so HAM reaches 8/8 before
        # the first real accumulation
        scratch = wpool.tile([128, 512], BF16, tag="scratch")
        nc.vector.memset(scratch[:], 0.0)
        wps = ps_s.tile([128, 512], F32, tag="ps_s")
        for _ in range(12):
            nc.tensor.matmul(wps[:], scratch[:, :128], scratch[:],
                             start=True, stop=True)

        # output projection units (one per (tok-tile, feat-block)); the
        # 1/Z normalization is fused into the PSUM->SBUF copy. These are
        # woven into later B phases so the 37 MB output DMA spreads over
        # the whole kernel instead of saturating the tail.
        fbs = [(0, 512), (512, 512), (1024, 512), (1536, 512), (2048, 256)]

        def proj_mm(t, fi, st):
            f0, fw = fbs[fi]
            pool = ps_qk if fi % 3 < 2 else ps_v
            ps = pool.tile([128, 512], F32, tag=pool.name)
            nc.tensor.matmul(ps[:, :fw], alo[:, t * 128:(t + 1) * 128],
                             wo[:, f0:f0 + fw], start=True, stop=False)
            nc.tensor.matmul(ps[:, :fw], ahi[:, t * 128:(t + 1) * 128],
                             wo[:, HID + f0:HID + f0 + fw],
                             start=False, stop=True)
            st[(t, fi)] = ps

        def proj_cp(t, fi, st, dve=False):
            f0, fw = fbs[fi]
            ps = st.pop((t, fi))
            osb = opool.tile([128, 512], BF16, tag="o")
            if fi % 2 == 0 and fi < 4 and not dve:
                nc.scalar.activation(osb[:, :fw], ps[:, :fw], AF.Copy,
                                     scale=rc[:, t:t + 1])
            else:
                nc.vector.tensor_scalar_mul(osb[:, :fw], ps[:, :fw],
                                            rc[:, t:t + 1])
            nc.sync.dma_start(out_d[t, fi, :, :fw], osb[:, :fw])

        def phase_c(qb, dve=False):
            # drain copy of proj i-1 rides with the matmuls of proj i, so
            # each copy has a full weave-slot of engine-queue slack before
            # its PSUM bank is recycled. The tail call (dve=True) keeps
            # every drain off the ACT queue, which is still finishing B7's
            # exps.
            prs = [(t, fi) for t in range(4 * qb, 4 * qb + 4)
                   for fi in range(len(fbs))]
            st = {}

            def unit(i):
                if i > 0:
                    proj_cp(*prs[i - 1], st, dve)
                proj_mm(*prs[i], st)

            units = [lambda i=i: unit(i) for i in range(len(prs))]
            units.append(lambda: proj_cp(*prs[-1], st, dve))
            return units

        def weave(bunits, aunits):
            """Alternate B and A units so stalled B consumers never block
            independent A matmuls in the in-order PE queue."""
            out = []
            na, nb = len(aunits), len(bunits)
            ai = 0
            for bi, bu in enumerate(bunits):
                out.append(bu)
                want = (bi + 1) * na // nb
                while ai < want:
                    out.append(aunits[ai])
                    ai += 1
            out.extend(aunits[ai:])
            return out

        # A runs two blocks ahead of B; each block's exchange-consume units
        # (recv DMAs + v transposes, the first PE-queue instructions that
        # WAIT on the collective) are woven a full B-phase after the
        # collective was issued, so the pair AllGather never stalls the
        # in-order PE queue.
        consumes = {}
        prod0, consumes[0] = phase_a(0)
        prod1, consumes[1] = phase_a(1)
        # prologue order: both xt DMAs first, A0 compute, then A0's
        # send+collective immediately (so cc_0 runs during A1 compute),
        # then A1, then consume(0)
        for u in ([prod0[0], prod0[1], prod1[0]] + prod0[2:]
                  + [prod1[1]] + prod1[2:] + consumes.pop(0)):
            u()
        for T in range(NT):
            bunits = phase_b(T)
            aunits = []
            if T + 2 < NT:
                prod, consumes[T + 2] = phase_a(T + 2)
                aunits += prod
            if T >= 1:
                aunits += phase_c(T - 1)
            if T + 1 in consumes:
                aunits += consumes.pop(T + 1)
            with nc.named_scope(f"B{T}"):
                for u in weave(bunits, aunits):
                    u()
        with nc.named_scope("Ctail"):
            for qb in (NT - 1,):
                for u in phase_c(qb, dve=True):
                    u()


    split_multi_waits(nc)
    return nc


def kernel(hidden_states, attention_mask, position_ids, Wqkv, Wo):
    bf16 = ml_dtypes.bfloat16
    hidden = np.asarray(hidden_states, np.float32)
    S = hidden.shape[1]
    X = hidden[0]  # [S, HID]
    XT = np.ascontiguousarray(X.T).astype(bf16)  # [HID, S]
    # block-major device layout: [128, (T, kc, s)] (see _build)
    XTB = np.ascontiguousarray(
        XT.reshape(KC, 128, S // 512, 512).transpose(1, 2, 0, 3)
        .reshape(128, (S // 512) * KC * 512))

    pos = np.asarray(position_ids)[0].astype(np.float64)
    inv = 1.0 / (ROPE_THETA ** (np.arange(0, HD, 2, dtype=np.float64) / HD))
    freqs = inv[:, None] * pos[None, :]  # [128, S]
    cosT = np.cos(freqs).astype(np.float32)
    sinT = np.sin(freqs).astype(np.float32)

    plan, maskb = _classify_mask(attention_mask, S)

    Wqkv = np.asarray(Wqkv, np.float32)
    Wo = np.asarray(Wo, np.float32)

    one_cs = np.ones_like(cosT)
    zero_cs = np.zeros_like(sinT)

    in_maps = []
    for c in range(N_CORES):
        g = c // (NH // NKV)
        wq = Wqkv[c * HD:(c + 1) * HD] * SCALE  # exact: SCALE = 2**-4
        wk = Wqkv[NH * HD + g * HD: NH * HD + (g + 1) * HD]
        wv = Wqkv[(NH + NKV) * HD + g * HD: (NH + NKV) * HD + (g + 1) * HD]
        # even core of a pair projects+sends k (RoPE'd), odd projects+sends
        # v (coss=1/sins=0 makes the s-RoPE the identity)
        ws = wk if c % 2 == 0 else wv
        wqk = np.ascontiguousarray(
            np.concatenate([wq.T, ws.T], axis=1)).astype(bf16)
        wot = np.ascontiguousarray(Wo[:, c * HD:(c + 1) * HD].T).astype(bf16)
        in_maps.append({
            "xt": XTB, "wqk": wqk, "wo": wot,
            "cosq": cosT, "sinq": sinT,
            "coss": cosT if c % 2 == 0 else one_cs,
            "sins": sinT if c % 2 == 0 else zero_cs,
            "maskb": maskb,
        })

    nc = _build(S, plan, maskb.shape[0])
    res = run_bass_kernel_spmd(nc, in_maps, list(range(N_CORES)),
                               trace=TRACE)
    acc = res.results[0]["out"].astype(np.float32)
    for c in range(1, N_CORES):
        acc += res.results[c]["out"].astype(np.float32)
    # unshuffle the [t, fi, p, 512] block layout back to [S, HID]
    fbs = [(0, 512), (512, 512), (1024, 512), (1536, 512), (2048, 256)]
    out = np.empty((S, HID), np.float32)
    for fi, (f0, fw) in enumerate(fbs):
        out[:, f0:f0 + fw] = acc[:, fi, :, :fw].reshape(S, fw)
    kernel.last_exec_time_ns = res.exec_time_ns
    kernel.last_results = res
    return out[None].astype(np.float32)


kernel.last_exec_time_ns = None
kernel.last_results = None

